# revision 31
# baseline (speedup 1.0000x reference)
"""BertImageSelfAttention Trainium2 kernel.

Shapes (fixed): hidden_states [4, 2048, 1024], 16 heads x 64, text [4, 64, 768].
Sharding: 8 cores = 4 batches x 2 head-groups (8 heads each). Each core computes
its batch's attention context for its 8 heads; host reassembles [4, 2048, 1024].

Per-core device pipeline (all matmuls bf16 with fp32 PSUM accumulation):
  A. pooled text -> dynamic Q/K gates (tiny matmuls + sigmoid)
  B. chunked loads: wk, wv, xT (4 s-blocks), wq on the SWDGE queue so the
     first KT projection chains start while the rest of xT streams in
  C. prefix projections: KT ec0 (interleaved with V per t-chunk as xT blocks
     land), QT ec0/sp0 -- just enough to start attention early; the remaining
     22 projection chains are interleaved into the attention groups' PE slack
     (deadline-scheduled so group g's inputs are always ready)
  D. 16 attention groups (h, sp): per t: S^T tile = K^T.T @ Q^T, ACT Exp
     (scale=1/8, bias=attention_mask), ctx^T[65,512] += Vaug.T @ E^T
     (ones column -> denominator in row 64); ctx lags scores by one t so the
     scalar engine (the bottleneck: 256 x ~1.3us Exp) never waits
  E. per group: evict ctx, pack the 4 denominator rows into one [8,128] tile,
     single full-lane reciprocal, broadcast via tiny DMAs, normalize + bias,
     DMA out [2048, 512] fp32 in ctx^T layout (host transposes).
"""

import os

import numpy as np
import ml_dtypes

import concourse.bass as bass
import concourse.bacc as bacc
import concourse.tile as tile
from concourse import mybir
from concourse.bass_utils import run_bass_kernel_spmd

P = 128
B, S, DV = 4, 2048, 1024
H, Dh = 16, 64
T, DT = 64, 768
NCORES = 8
E = 512          # head-group width (8 heads x 64)
CC = DV // P     # 8 contraction chunks for projections
ECH = E // P     # 4 e-chunks
DC = DT // P     # 6 text-dim chunks
SC = S // P      # 16 seq chunks of 128
SBL = S // 512   # 4 seq blocks of 512
HPC = 8          # heads per core

FP32 = mybir.dt.float32
BF16 = mybir.dt.bfloat16
AF = mybir.ActivationFunctionType
OP = mybir.AluOpType

BF16_NP = ml_dtypes.bfloat16

_CACHE = {}

# module-level stash of the last BassKernelResults (for test.py introspection)
last_results = None


def _emit(tc, aps):
    nc = tc.nc
    # all inputs arrive pre-permuted from the host so every DMA reads
    # contiguous per-partition lines (the lowered "(c p) -> p c" scatter
    # patterns measured 4-40x slower than contiguous loads)
    xT = aps["xT"]                                             # [128, 8, 2048]
    wq = aps["wq"]                                             # [128, 8, 512]
    wk = aps["wk"]
    wv = aps["wv"]
    wdq = aps["wdq"]                                           # [128, 6, 512]
    wdk = aps["wdk"]
    txt = aps["txt"]                                           # [64, 768] bf16
    tmask = aps["tmask"]                                       # [64, 1] bf16
    amask = aps["amask"]                                       # [128, 16]
    bq = aps["bq"]                                             # [128, 4]
    bk = aps["bk"]
    bdq = aps["bdq"]
    bdk = aps["bdk"]
    bv = aps["bv"]                                             # [128, 4] (bvT)
    out = aps["out"]                                           # [8, 64, 2048] f32

    from contextlib import ExitStack

    with ExitStack() as ctx:
        wpool = ctx.enter_context(tc.tile_pool(name="wpool", bufs=1))
        xpool = ctx.enter_context(tc.tile_pool(name="xpool", bufs=1))
        qkpool = ctx.enter_context(tc.tile_pool(name="qkpool", bufs=1))
        vpool = ctx.enter_context(tc.tile_pool(name="vpool", bufs=1))
        etp = ctx.enter_context(tc.tile_pool(name="etp", bufs=8))
        rbp = ctx.enter_context(tc.tile_pool(name="rbp", bufs=6))
        outp = ctx.enter_context(tc.tile_pool(name="outp", bufs=4))
        smallp = ctx.enter_context(tc.tile_pool(name="smallp", bufs=1))
        dkp = ctx.enter_context(tc.tile_pool(name="dkp", bufs=4))
        # PSUM: 8 banks = scp 2 x [128,1024] (4) + ctxp 2 x [128,512] (2)
        # + projp 2 x [128,512] (2).  projp hosts the interleaved projection
        # chains (and the gate matmuls in the prefix).
        ctxp = ctx.enter_context(tc.tile_pool(name="ctxp", bufs=2, space="PSUM"))
        projp = ctx.enter_context(tc.tile_pool(name="projp", bufs=2, space="PSUM"))
        scp = ctx.enter_context(tc.tile_pool(name="scp", bufs=2, space="PSUM"))

        # text tensors padded to 128 partitions (zero rows 64..127) so every
        # matmul runs in uniform (128,128) PE tile mode — no mode switches.
        txt_sb = smallp.tile([P, DT], BF16, tag="txt")
        nc.vector.memset(txt_sb[T:P, :], 0.0)
        nc.sync.dma_start(out=txt_sb[0:T, :], in_=txt)
        # mask as a [128,128] stationary: column 0 = mask, rest zero -> M=128
        tmask_sb = smallp.tile([P, P], BF16, tag="tmask")
        nc.vector.memset(tmask_sb, 0.0)
        nc.sync.dma_start(out=tmask_sb[0:T, 0:1], in_=tmask)
        ones_sb = smallp.tile([P, 1], BF16, tag="ones")
        nc.vector.memset(ones_sb, 1.0)
        amask_sb = smallp.tile([P, SC], FP32, tag="amask")
        nc.sync.dma_start(out=amask_sb, in_=amask)
        bq_sb = smallp.tile([P, ECH], FP32, tag="bq")
        nc.sync.dma_start(out=bq_sb, in_=bq)
        bk_sb = smallp.tile([P, ECH], FP32, tag="bk")
        nc.sync.dma_start(out=bk_sb, in_=bk)
        bdq_sb = smallp.tile([P, ECH], FP32, tag="bdq")
        nc.sync.dma_start(out=bdq_sb, in_=bdq)
        bdk_sb = smallp.tile([P, ECH], FP32, tag="bdk")
        nc.sync.dma_start(out=bdk_sb, in_=bdk)
        bvT_sb = smallp.tile([P, ECH], FP32, tag="bvT")
        nc.sync.dma_start(out=bvT_sb, in_=bv)

        # ---- big loads, split across all three DMA queues so the critical
        # pieces land in parallel: SWDGE carries KT/QT ec0 inputs (wk, xT
        # blocks 0-1, wq), the scalar HWDGE queue carries wv + xT blocks 2-3,
        # and the sync queue (after the smalls) carries the gate weights ----
        wdq_sb = wpool.tile([P, DC, E], BF16, tag="wdq")
        nc.sync.dma_start(out=wdq_sb, in_=wdq)
        wdk_sb = wpool.tile([P, DC, E], BF16, tag="wdk")
        nc.sync.dma_start(out=wdk_sb, in_=wdk)
        wk_sb = wpool.tile([P, CC, E], BF16, tag="wk")
        nc.gpsimd.dma_start(out=wk_sb, in_=wk)
        xT_sb = xpool.tile([P, CC, S], BF16, tag="xT")
        nc.gpsimd.dma_start(out=xT_sb[:, :, 0:512], in_=xT[:, :, 0:512])
        wq_sb = wpool.tile([P, CC, E], BF16, tag="wq")
        nc.gpsimd.dma_start(out=wq_sb, in_=wq)
        nc.gpsimd.dma_start(out=xT_sb[:, :, 512:1024], in_=xT[:, :, 512:1024])
        wv_sb = wpool.tile([P, CC, E], BF16, tag="wv")
        nc.scalar.dma_start(out=wv_sb, in_=wv)
        nc.scalar.dma_start(out=xT_sb[:, :, 1024:1536], in_=xT[:, :, 1024:1536])
        nc.scalar.dma_start(out=xT_sb[:, :, 1536:2048], in_=xT[:, :, 1536:2048])

        # ---- gates: pooled text + sigmoid ----
        pr = scp.tile([P, 769], FP32, tag="sc")
        nc.tensor.matmul(pr[:, 0:512], lhsT=tmask_sb, rhs=txt_sb[:, 0:512],
                         start=True, stop=True)
        nc.tensor.matmul(pr[:, 512:768], lhsT=tmask_sb, rhs=txt_sb[:, 512:768],
                         start=True, stop=True)
        nc.tensor.matmul(pr[:, 768:769], lhsT=tmask_sb, rhs=ones_sb,
                         start=True, stop=True)
        rmsum = smallp.tile([1, 1], FP32, tag="rmsum")
        nc.vector.reciprocal(rmsum, pr[0:1, 768:769])
        prow = smallp.tile([1, DT], BF16, tag="prow")
        nc.vector.tensor_scalar(prow, pr[0:1, 0:768], rmsum, None, OP.mult)

        # scatter pooled row -> poolT [128, 6] (dt on partitions) via tiny
        # SBUF->SBUF DMA (dt = c*128 + p), spread over three DMA queues
        poolT = smallp.tile([P, DC], BF16, tag="poolT")
        for c in range(DC):
            dq = (nc.sync, nc.scalar)[c % 2]
            dq.dma_start(
                out=poolT[:, c:c + 1],
                in_=prow[0:1, c * P:(c + 1) * P],
            )

        # gates: g = 1 + sigmoid(pool @ Wd + bd); also g*b for fused bias
        gq_sb = smallp.tile([P, ECH], FP32, tag="gq")
        gk_sb = smallp.tile([P, ECH], FP32, tag="gk")
        gbq_sb = smallp.tile([P, ECH], FP32, tag="gbq")
        gbk_sb = smallp.tile([P, ECH], FP32, tag="gbk")
        # the softmax 1/sqrt(Dh)=0.125 scale is folded into the Q gate so the
        # Exp activation needs no scale operand: g_q = 0.125*(1+sigmoid(...))
        for (wd_sb, bd_sb, b_sb, g_sb, gb_sb, gsc) in (
            (wdq_sb, bdq_sb, bq_sb, gq_sb, gbq_sb, 0.125),
            (wdk_sb, bdk_sb, bk_sb, gk_sb, gbk_sb, 1.0),
        ):
            for ec in range(ECH):
                gp = projp.tile([P, 512], FP32, tag="proj")
                for c in range(DC):
                    nc.tensor.matmul(
                        gp[:, 0:1],
                        lhsT=wd_sb[:, c, ec * P:(ec + 1) * P],
                        rhs=poolT[:, c:c + 1],
                        start=(c == 0), stop=(c == DC - 1),
                    )
                nc.scalar.activation(g_sb[:, ec:ec + 1], gp[:, 0:1], AF.Sigmoid,
                                     bias=bd_sb[:, ec:ec + 1])
            nc.vector.tensor_scalar(g_sb, g_sb, gsc, gsc, OP.mult, OP.add)
            nc.vector.tensor_mul(gb_sb, g_sb, b_sb)

        # exp(mask) folded into V and the denominator column:
        # exp(s/8 + m[k]) = exp(s/8) * em[k], and em scales per KEY = per
        # partition of Vaug, so the Exp activation needs no bias operand.
        em_sb = smallp.tile([P, SC], FP32, tag="em")
        nc.scalar.activation(em_sb, amask_sb, AF.Exp)
        ones8 = smallp.tile([P, HPC, 1], BF16, tag="ones8")
        nc.vector.memset(ones8, 1.0)
        ones64 = smallp.tile([P, Dh], BF16, tag="ones64")
        nc.vector.memset(ones64, 1.0)

        # ---- persistent SBUF tensors for projections ----
        Vaug = vpool.tile([P, SC, HPC, Dh + 1], BF16, tag="Vaug")
        QT = qkpool.tile([P, ECH, S], BF16, tag="QT")
        KTp = qkpool.tile([P, HPC, S], BF16, tag="KTp")

        # ---- projection chain emitters (8 accumulation MMs + eviction) ----
        def v_chain(t):
            ps = projp.tile([P, 512], FP32, tag="proj", name=f"psv{t}")
            for c in range(CC):
                nc.tensor.matmul(
                    ps,
                    lhsT=xT_sb[:, c, t * P:(t + 1) * P],
                    rhs=wv_sb[:, c, :],
                    start=(c == 0), stop=(c == CC - 1),
                )
            nc.vector.tensor_scalar(
                Vaug[:, t, :, 0:Dh],
                ps.rearrange("p (h d) -> p h d", h=HPC),
                em_sb[:, t:t + 1], None, OP.mult,
            )
            nc.vector.tensor_scalar(
                Vaug[:, t, :, Dh:Dh + 1], ones8,
                em_sb[:, t:t + 1], None, OP.mult,
            )

        def qt_chain(ec, ss):
            sl = slice(ss * 512, (ss + 1) * 512)
            ps = projp.tile([P, 512], FP32, tag="proj", name=f"psq{ec}_{ss}")
            for c in range(CC):
                nc.tensor.matmul(
                    ps,
                    lhsT=wq_sb[:, c, ec * P:(ec + 1) * P],
                    rhs=xT_sb[:, c, sl],
                    start=(c == 0), stop=(c == CC - 1),
                )
            # (x@W)*g + g*b fused into eviction, cast bf16
            nc.vector.tensor_scalar(
                QT[:, ec, sl], ps,
                gq_sb[:, ec:ec + 1], gbq_sb[:, ec:ec + 1],
                OP.mult, OP.add,
            )

        def kt_chain(ec, ss):
            sl = slice(ss * 512, (ss + 1) * 512)
            ps = projp.tile([P, 512], FP32, tag="proj", name=f"psk{ec}_{ss}")
            for c in range(CC):
                nc.tensor.matmul(
                    ps,
                    lhsT=wk_sb[:, c, ec * P:(ec + 1) * P],
                    rhs=xT_sb[:, c, sl],
                    start=(c == 0), stop=(c == CC - 1),
                )
            # per-head zero-padded to full 128 partitions so score matmuls
            # contract K=128 in the same (128,128) mode as the rest; the pad
            # halves are zeroed here per-chain (cheap bf16 DVE memsets that
            # overlap compute) instead of one giant up-front memset that
            # stalled the first evictions ~16us
            for hi in range(2):
                pp = slice(hi * Dh, (hi + 1) * Dh)
                po = slice((1 - hi) * Dh, (2 - hi) * Dh)
                nc.vector.tensor_scalar(
                    KTp[pp, 2 * ec + hi, sl], ps[pp, :],
                    gk_sb[pp, ec:ec + 1], gbk_sb[pp, ec:ec + 1],
                    OP.mult, OP.add,
                )
                nc.vector.memset(KTp[po, 2 * ec + hi, sl], 0.0)

        # ---- prefix: the minimum projection for the first exps ----
        kt_chain(0, 0)
        qt_chain(0, 0)
        qt_chain(0, 1)

        # remaining chains, interleaved into attention groups' PE slack.
        # group order: heads of one e-chunk first at sp0, then their sp1, so
        # each (KT ec, QT ec/sp) chain is needed as late as possible.
        # group 0 carries the V chains (V t-chunk t is only needed by its
        # lagged ctx matmul) plus KT ec0's remaining s-blocks.
        groups = [(0, 0), (1, 0), (0, 1), (1, 1),
                  (2, 0), (3, 0), (2, 1), (3, 1),
                  (4, 0), (5, 0), (4, 1), (5, 1),
                  (6, 0), (7, 0), (6, 1), (7, 1)]
        # per-group: {iter: [chains to emit after that iter's scores]}
        g0 = {}
        for t in range(8):
            g0[t] = [(lambda tt=2 * t: v_chain(tt)),
                     (lambda tt=2 * t + 1: v_chain(tt))]
        g0[1].append(lambda: kt_chain(0, 1))
        g0[3].append(lambda: kt_chain(0, 2))
        g0[5].append(lambda: kt_chain(0, 3))
        fillers = {
            0: g0,
            1: {2: [lambda: qt_chain(0, 2)], 8: [lambda: qt_chain(0, 3)]},
            2: {2: [lambda: kt_chain(1, 0)], 7: [lambda: kt_chain(1, 1)],
                12: [lambda: kt_chain(1, 2)]},
            3: {2: [lambda: kt_chain(1, 3)], 7: [lambda: qt_chain(1, 0)],
                12: [lambda: qt_chain(1, 1)]},
            4: {4: [lambda: qt_chain(1, 2)]},
            5: {4: [lambda: qt_chain(1, 3)], 10: [lambda: kt_chain(2, 0)]},
            6: {4: [lambda: kt_chain(2, 1)], 10: [lambda: kt_chain(2, 2)]},
            7: {2: [lambda: kt_chain(2, 3)], 7: [lambda: qt_chain(2, 0)],
                12: [lambda: qt_chain(2, 1)]},
            8: {4: [lambda: qt_chain(2, 2)]},
            9: {4: [lambda: qt_chain(2, 3)], 10: [lambda: kt_chain(3, 0)]},
            10: {4: [lambda: kt_chain(3, 1)], 10: [lambda: kt_chain(3, 2)]},
            11: {2: [lambda: kt_chain(3, 3)], 7: [lambda: qt_chain(3, 0)],
                 12: [lambda: qt_chain(3, 1)]},
            12: {4: [lambda: qt_chain(3, 2)]},
            13: {4: [lambda: qt_chain(3, 3)]},
        }

        # ---- attention groups ----
        for g, (h, sp) in enumerate(groups):
            hp, hi = h // 2, h % 2
            fill = fillers.get(g, {})
            ctx_ps = [ctxp.tile([P, 512], FP32, tag="ctx",
                                name=f"ctx{g}_{k}") for k in range(2)]
            ets = {}
            for t in range(SC):
                sps = scp.tile([P, 1024], FP32, tag="sc")
                for j in range(2):
                    q0 = sp * 1024 + j * 512
                    nc.tensor.matmul(
                        sps[:, j * 512:(j + 1) * 512],
                        lhsT=KTp[:, h, t * P:(t + 1) * P],
                        rhs=QT[:, hp, q0:q0 + 512],
                        start=True, stop=True,
                    )
                et = etp.tile([P, 1024], BF16, tag="et")
                nc.scalar.activation(et, sps, AF.Exp)
                ets[t] = et
                # ctx lags scores by one t so PE always has queued work while
                # ACT exponentiates, and exp(t) is never gated on ctx
                if t > 0:
                    for j in range(2):
                        nc.tensor.matmul(
                            ctx_ps[j][0:Dh + 1, :],
                            lhsT=Vaug[:, t - 1, h, :],
                            rhs=ets[t - 1][:, j * 512:(j + 1) * 512],
                            start=(t - 1 == 0), stop=False,
                        )
                    del ets[t - 1]
                for chain in fill.get(t, ()):
                    chain()
            for j in range(2):
                nc.tensor.matmul(
                    ctx_ps[j][0:Dh + 1, :],
                    lhsT=Vaug[:, SC - 1, h, :],
                    rhs=ets[SC - 1][:, j * 512:(j + 1) * 512],
                    start=False, stop=True,
                )

            # ---- normalize + output ----
            # evict PSUM immediately; denominators (row 64 of each j block)
            # are packed onto 32-aligned partitions so (a) the reciprocal
            # runs multi-lane and (b) tiny K=1 ones-stationary matmuls
            # replicate 1/denom across 64 partitions straight into PSUM —
            # no high-latency broadcast DMAs on the tail's critical path.
            cs = []
            for j in range(2):
                c_t = rbp.tile([Dh + 1, 512], FP32, tag="cs")
                nc.vector.tensor_copy(c_t, ctx_ps[j][0:Dh + 1, :])
                cs.append(c_t)
            dpk = dkp.tile([8, P], FP32, tag="dpk")
            for j in range(2):
                nc.sync.dma_start(out=dpk[4 * j:4 * j + 4, :],
                                  in_=cs[j][Dh:Dh + 1, :])
            rct = dkp.tile([8, P], BF16, tag="rct")
            with nc.allow_low_precision(reason="1/denom cast bf16 for PE bcast"):
                nc.vector.reciprocal(rct, dpk)
            for j in range(2):
                sb = sp * 2 + j
                # spread 1/denom to partitions {0,32}, then two K=1 matmuls
                # with a ones stationary replicate it across 64 partitions
                # straight into PSUM (no high-latency broadcast DMAs)
                rca = dkp.tile([33, 256], BF16, tag="rca", name=f"rca{g}_{j}")
                nc.sync.dma_start(out=rca[0:1, :], in_=rct[4 * j:4 * j + 2, :])
                nc.sync.dma_start(out=rca[32:33, :], in_=rct[4 * j + 2:4 * j + 4, :])
                rcb_ps = projp.tile([Dh, 512], FP32, tag="proj",
                                    name=f"rcb{g}_{j}")
                for q in range(2):
                    nc.tensor.matmul(
                        rcb_ps[:, q * 256:(q + 1) * 256],
                        lhsT=ones64[32 * q:32 * q + 1, :],
                        rhs=rca[32 * q:32 * q + 1, :],
                        start=True, stop=True,
                    )
                ot = outp.tile([Dh, 512], FP32, tag="outsb")
                nc.vector.tensor_mul(ot, cs[j][0:Dh, :], rcb_ps)
                nc.vector.tensor_scalar(
                    ot, ot,
                    bvT_sb[hi * Dh:(hi + 1) * Dh, hp:hp + 1], None,
                    OP.add,
                )
                nc.sync.dma_start(
                    out=out[h, :, sb * 512:(sb + 1) * 512], in_=ot)


def _build():
    key = "nc"
    if key in _CACHE:
        return _CACHE[key]
    nc = bacc.Bacc("TRN2", target_bir_lowering=False, debug=False,
                   enable_asserts=False)
    aps = {}

    def din(name, shape, dt):
        aps[name] = nc.dram_tensor(name, shape, dt, kind="ExternalInput").ap()

    din("xT", [P, CC, S], BF16)
    din("wq", [P, CC, E], BF16)
    din("wk", [P, CC, E], BF16)
    din("wv", [P, CC, E], BF16)
    din("wdq", [P, DC, E], BF16)
    din("wdk", [P, DC, E], BF16)
    din("txt", [T, DT], BF16)
    din("tmask", [T, 1], BF16)
    din("amask", [P, SC], FP32)
    din("bq", [P, ECH], FP32)
    din("bk", [P, ECH], FP32)
    din("bv", [P, ECH], FP32)
    din("bdq", [P, ECH], FP32)
    din("bdk", [P, ECH], FP32)
    aps["out"] = nc.dram_tensor("out", [HPC, Dh, S], FP32,
                                kind="ExternalOutput").ap()

    with tile.TileContext(nc) as tc:
        _emit(tc, aps)
    nc.compile()
    _CACHE[key] = nc
    return nc


def kernel(**inputs):
    global last_results
    hs = np.asarray(inputs["hidden_states"], dtype=np.float32)
    amask = np.asarray(inputs["attention_mask"], dtype=np.float32)
    txt = np.asarray(inputs["txt_embedding"], dtype=np.float32)
    tmask = np.asarray(inputs["txt_attention_mask"], dtype=np.float32)
    Wq = np.asarray(inputs["Wq"], dtype=np.float32)
    Wk = np.asarray(inputs["Wk"], dtype=np.float32)
    Wv = np.asarray(inputs["Wv"], dtype=np.float32)
    Wdq = np.asarray(inputs["Wdq"], dtype=np.float32)
    Wdk = np.asarray(inputs["Wdk"], dtype=np.float32)
    bq = np.asarray(inputs["bq"], dtype=np.float32)
    bk = np.asarray(inputs["bk"], dtype=np.float32)
    bv = np.asarray(inputs["bv"], dtype=np.float32)
    bdq = np.asarray(inputs["bdq"], dtype=np.float32)
    bdk = np.asarray(inputs["bdk"], dtype=np.float32)

    nc = _build()

    def pcs(w):
        # [D, e] -> [128, D//128, e]: partition-major so each partition's
        # DMA line is one contiguous block
        d = w.shape[0]
        return np.ascontiguousarray(
            w.reshape(d // P, P, w.shape[1]).transpose(1, 0, 2))

    def pvec(v):
        # [k*128] -> [128, k]
        return np.ascontiguousarray(v.reshape(-1, P).T)

    in_maps = []
    for c in range(NCORES):
        b, g = c // 2, c % 2
        cols = slice(g * E, (g + 1) * E)
        in_maps.append({
            "xT": pcs(np.ascontiguousarray(hs[b].T)).astype(BF16_NP),
            "wq": pcs(Wq[:, cols]).astype(BF16_NP),
            "wk": pcs(Wk[:, cols]).astype(BF16_NP),
            "wv": pcs(Wv[:, cols]).astype(BF16_NP),
            "wdq": pcs(Wdq[:, cols]).astype(BF16_NP),
            "wdk": pcs(Wdk[:, cols]).astype(BF16_NP),
            "txt": txt[b].astype(BF16_NP),
            "tmask": tmask[b].astype(BF16_NP),
            "amask": pvec(amask[b, 0, 0]),
            "bq": pvec(bq[cols]),
            "bk": pvec(bk[cols]),
            "bv": pvec(bv[cols]),
            "bdq": pvec(bdq[cols]),
            "bdk": pvec(bdk[cols]),
        })

    tr = int(os.environ.get("BASS_KERNEL_TRACE", "0"))
    if tr == 2:
        # warm the NEFF (compile+load+run untraced), then trace a second run
        run_bass_kernel_spmd(nc, in_maps, list(range(NCORES)), trace=False)
    res = run_bass_kernel_spmd(nc, in_maps, list(range(NCORES)), trace=bool(tr))
    last_results = res

    outp = np.empty((B, S, DV), dtype=np.float32)
    for c in range(NCORES):
        b, g = c // 2, c % 2
        # device output is ctx^T [head, d, s] -> [s, head*64+d]
        co = res.results[c]["out"].transpose(2, 0, 1).reshape(S, E)
        outp[b, :, g * E:(g + 1) * E] = co
    return outp


# revision 32
# speedup vs baseline: 1.0035x; 1.0035x over previous
"""BertImageSelfAttention Trainium2 kernel.

Shapes (fixed): hidden_states [4, 2048, 1024], 16 heads x 64, text [4, 64, 768].
Sharding: 8 cores = 4 batches x 2 head-groups (8 heads each). Each core computes
its batch's attention context for its 8 heads; host reassembles [4, 2048, 1024].

Per-core device pipeline (all matmuls bf16 with fp32 PSUM accumulation):
  A. pooled text -> dynamic Q/K gates (tiny matmuls + sigmoid)
  B. chunked loads: wk, wv, xT (4 s-blocks), wq on the SWDGE queue so the
     first KT projection chains start while the rest of xT streams in
  C. prefix projections: KT ec0 (interleaved with V per t-chunk as xT blocks
     land), QT ec0/sp0 -- just enough to start attention early; the remaining
     22 projection chains are interleaved into the attention groups' PE slack
     (deadline-scheduled so group g's inputs are always ready)
  D. 16 attention groups (h, sp): per t: S^T tile = K^T.T @ Q^T, ACT Exp
     (scale=1/8, bias=attention_mask), ctx^T[65,512] += Vaug.T @ E^T
     (ones column -> denominator in row 64); ctx lags scores by one t so the
     scalar engine (the bottleneck: 256 x ~1.3us Exp) never waits
  E. per group: evict ctx, pack the 4 denominator rows into one [8,128] tile,
     single full-lane reciprocal, broadcast via tiny DMAs, normalize + bias,
     DMA out [2048, 512] fp32 in ctx^T layout (host transposes).
"""

import os

import numpy as np
import ml_dtypes

import concourse.bass as bass
import concourse.bacc as bacc
import concourse.tile as tile
from concourse import mybir
from concourse.bass_utils import run_bass_kernel_spmd

P = 128
B, S, DV = 4, 2048, 1024
H, Dh = 16, 64
T, DT = 64, 768
NCORES = 8
E = 512          # head-group width (8 heads x 64)
CC = DV // P     # 8 contraction chunks for projections
ECH = E // P     # 4 e-chunks
DC = DT // P     # 6 text-dim chunks
SC = S // P      # 16 seq chunks of 128
SBL = S // 512   # 4 seq blocks of 512
HPC = 8          # heads per core

FP32 = mybir.dt.float32
BF16 = mybir.dt.bfloat16
AF = mybir.ActivationFunctionType
OP = mybir.AluOpType

BF16_NP = ml_dtypes.bfloat16

_CACHE = {}

# module-level stash of the last BassKernelResults (for test.py introspection)
last_results = None


def _emit(tc, aps):
    nc = tc.nc
    # all inputs arrive pre-permuted from the host so every DMA reads
    # contiguous per-partition lines (the lowered "(c p) -> p c" scatter
    # patterns measured 4-40x slower than contiguous loads)
    xT = aps["xT"]                                             # [128, 8, 2048]
    wq = aps["wq"]                                             # [128, 8, 512]
    wk = aps["wk"]
    wv = aps["wv"]
    wdq = aps["wdq"]                                           # [128, 6, 512]
    wdk = aps["wdk"]
    txt = aps["txt"]                                           # [64, 768] bf16
    tmask = aps["tmask"]                                       # [64, 1] bf16
    amask = aps["amask"]                                       # [128, 16]
    bq = aps["bq"]                                             # [128, 4]
    bk = aps["bk"]
    bdq = aps["bdq"]
    bdk = aps["bdk"]
    bv = aps["bv"]                                             # [128, 4] (bvT)
    out = aps["out"]                                           # [8, 64, 2048] f32

    from contextlib import ExitStack

    with ExitStack() as ctx:
        wpool = ctx.enter_context(tc.tile_pool(name="wpool", bufs=1))
        xpool = ctx.enter_context(tc.tile_pool(name="xpool", bufs=1))
        qkpool = ctx.enter_context(tc.tile_pool(name="qkpool", bufs=1))
        vpool = ctx.enter_context(tc.tile_pool(name="vpool", bufs=1))
        etp = ctx.enter_context(tc.tile_pool(name="etp", bufs=8))
        rbp = ctx.enter_context(tc.tile_pool(name="rbp", bufs=5))
        outp = ctx.enter_context(tc.tile_pool(name="outp", bufs=4))
        smallp = ctx.enter_context(tc.tile_pool(name="smallp", bufs=1))
        dkp = ctx.enter_context(tc.tile_pool(name="dkp", bufs=8))
        # PSUM: 8 banks = scp 2 x [128,1024] (4) + ctxp 2 x [128,512] (2)
        # + projp 2 x [128,512] (2).  projp hosts the interleaved projection
        # chains (and the gate matmuls in the prefix).
        ctxp = ctx.enter_context(tc.tile_pool(name="ctxp", bufs=2, space="PSUM"))
        projp = ctx.enter_context(tc.tile_pool(name="projp", bufs=2, space="PSUM"))
        scp = ctx.enter_context(tc.tile_pool(name="scp", bufs=2, space="PSUM"))

        # text tensors padded to 128 partitions (zero rows 64..127) so every
        # matmul runs in uniform (128,128) PE tile mode — no mode switches.
        txt_sb = smallp.tile([P, DT], BF16, tag="txt")
        nc.vector.memset(txt_sb[T:P, :], 0.0)
        nc.sync.dma_start(out=txt_sb[0:T, :], in_=txt)
        # mask as a [128,128] stationary: column 0 = mask, rest zero -> M=128
        tmask_sb = smallp.tile([P, P], BF16, tag="tmask")
        nc.vector.memset(tmask_sb, 0.0)
        nc.sync.dma_start(out=tmask_sb[0:T, 0:1], in_=tmask)
        ones_sb = smallp.tile([P, 1], BF16, tag="ones")
        nc.vector.memset(ones_sb, 1.0)
        amask_sb = smallp.tile([P, SC], FP32, tag="amask")
        nc.sync.dma_start(out=amask_sb, in_=amask)
        bq_sb = smallp.tile([P, ECH], FP32, tag="bq")
        nc.sync.dma_start(out=bq_sb, in_=bq)
        bk_sb = smallp.tile([P, ECH], FP32, tag="bk")
        nc.sync.dma_start(out=bk_sb, in_=bk)
        bdq_sb = smallp.tile([P, ECH], FP32, tag="bdq")
        nc.sync.dma_start(out=bdq_sb, in_=bdq)
        bdk_sb = smallp.tile([P, ECH], FP32, tag="bdk")
        nc.sync.dma_start(out=bdk_sb, in_=bdk)
        bvT_sb = smallp.tile([P, ECH], FP32, tag="bvT")
        nc.sync.dma_start(out=bvT_sb, in_=bv)

        # ---- big loads, split across all three DMA queues so the critical
        # pieces land in parallel: SWDGE carries KT/QT ec0 inputs (wk, xT
        # blocks 0-1, wq), the scalar HWDGE queue carries wv + xT blocks 2-3,
        # and the sync queue (after the smalls) carries the gate weights ----
        wdq_sb = wpool.tile([P, DC, E], BF16, tag="wdq")
        nc.sync.dma_start(out=wdq_sb, in_=wdq)
        wdk_sb = wpool.tile([P, DC, E], BF16, tag="wdk")
        nc.sync.dma_start(out=wdk_sb, in_=wdk)
        wk_sb = wpool.tile([P, CC, E], BF16, tag="wk")
        nc.gpsimd.dma_start(out=wk_sb, in_=wk)
        xT_sb = xpool.tile([P, CC, S], BF16, tag="xT")
        nc.gpsimd.dma_start(out=xT_sb[:, :, 0:512], in_=xT[:, :, 0:512])
        wq_sb = wpool.tile([P, CC, E], BF16, tag="wq")
        nc.gpsimd.dma_start(out=wq_sb, in_=wq)
        nc.gpsimd.dma_start(out=xT_sb[:, :, 512:1024], in_=xT[:, :, 512:1024])
        wv_sb = wpool.tile([P, CC, E], BF16, tag="wv")
        nc.scalar.dma_start(out=wv_sb, in_=wv)
        nc.scalar.dma_start(out=xT_sb[:, :, 1024:1536], in_=xT[:, :, 1024:1536])
        nc.scalar.dma_start(out=xT_sb[:, :, 1536:2048], in_=xT[:, :, 1536:2048])

        # ---- gates: pooled text + sigmoid ----
        pr = scp.tile([P, 769], FP32, tag="sc")
        nc.tensor.matmul(pr[:, 0:512], lhsT=tmask_sb, rhs=txt_sb[:, 0:512],
                         start=True, stop=True)
        nc.tensor.matmul(pr[:, 512:768], lhsT=tmask_sb, rhs=txt_sb[:, 512:768],
                         start=True, stop=True)
        nc.tensor.matmul(pr[:, 768:769], lhsT=tmask_sb, rhs=ones_sb,
                         start=True, stop=True)
        rmsum = smallp.tile([1, 1], FP32, tag="rmsum")
        nc.vector.reciprocal(rmsum, pr[0:1, 768:769])
        prow = smallp.tile([1, DT], BF16, tag="prow")
        nc.vector.tensor_scalar(prow, pr[0:1, 0:768], rmsum, None, OP.mult)

        # scatter pooled row -> poolT [128, 6] (dt on partitions) via tiny
        # SBUF->SBUF DMA (dt = c*128 + p), spread over three DMA queues
        poolT = smallp.tile([P, DC], BF16, tag="poolT")
        for c in range(DC):
            dq = (nc.sync, nc.scalar)[c % 2]
            dq.dma_start(
                out=poolT[:, c:c + 1],
                in_=prow[0:1, c * P:(c + 1) * P],
            )

        # gates: g = 1 + sigmoid(pool @ Wd + bd); also g*b for fused bias
        gq_sb = smallp.tile([P, ECH], FP32, tag="gq")
        gk_sb = smallp.tile([P, ECH], FP32, tag="gk")
        gbq_sb = smallp.tile([P, ECH], FP32, tag="gbq")
        gbk_sb = smallp.tile([P, ECH], FP32, tag="gbk")
        # the softmax 1/sqrt(Dh)=0.125 scale is folded into the Q gate so the
        # Exp activation needs no scale operand: g_q = 0.125*(1+sigmoid(...))
        for (wd_sb, bd_sb, b_sb, g_sb, gb_sb, gsc) in (
            (wdq_sb, bdq_sb, bq_sb, gq_sb, gbq_sb, 0.125),
            (wdk_sb, bdk_sb, bk_sb, gk_sb, gbk_sb, 1.0),
        ):
            for ec in range(ECH):
                gp = projp.tile([P, 512], FP32, tag="proj")
                for c in range(DC):
                    nc.tensor.matmul(
                        gp[:, 0:1],
                        lhsT=wd_sb[:, c, ec * P:(ec + 1) * P],
                        rhs=poolT[:, c:c + 1],
                        start=(c == 0), stop=(c == DC - 1),
                    )
                nc.scalar.activation(g_sb[:, ec:ec + 1], gp[:, 0:1], AF.Sigmoid,
                                     bias=bd_sb[:, ec:ec + 1])
            nc.vector.tensor_scalar(g_sb, g_sb, gsc, gsc, OP.mult, OP.add)
            nc.vector.tensor_mul(gb_sb, g_sb, b_sb)

        # exp(mask) folded into V and the denominator column:
        # exp(s/8 + m[k]) = exp(s/8) * em[k], and em scales per KEY = per
        # partition of Vaug, so the Exp activation needs no bias operand.
        em_sb = smallp.tile([P, SC], FP32, tag="em")
        nc.scalar.activation(em_sb, amask_sb, AF.Exp)
        ones8 = smallp.tile([P, HPC, 1], BF16, tag="ones8")
        nc.vector.memset(ones8, 1.0)
        ones64 = smallp.tile([P, Dh], BF16, tag="ones64")
        nc.vector.memset(ones64, 1.0)

        # ---- persistent SBUF tensors for projections ----
        Vaug = vpool.tile([P, SC, HPC, Dh + 1], BF16, tag="Vaug")
        QT = qkpool.tile([P, ECH, S], BF16, tag="QT")
        KTp = qkpool.tile([P, HPC, S], BF16, tag="KTp")

        # ---- projection chain emitters (8 accumulation MMs + eviction) ----
        def v_chain(t):
            ps = projp.tile([P, 512], FP32, tag="proj", name=f"psv{t}")
            for c in range(CC):
                nc.tensor.matmul(
                    ps,
                    lhsT=xT_sb[:, c, t * P:(t + 1) * P],
                    rhs=wv_sb[:, c, :],
                    start=(c == 0), stop=(c == CC - 1),
                )
            nc.vector.tensor_scalar(
                Vaug[:, t, :, 0:Dh],
                ps.rearrange("p (h d) -> p h d", h=HPC),
                em_sb[:, t:t + 1], None, OP.mult,
            )
            nc.vector.tensor_scalar(
                Vaug[:, t, :, Dh:Dh + 1], ones8,
                em_sb[:, t:t + 1], None, OP.mult,
            )

        def qt_chain(ec, ss):
            sl = slice(ss * 512, (ss + 1) * 512)
            ps = projp.tile([P, 512], FP32, tag="proj", name=f"psq{ec}_{ss}")
            for c in range(CC):
                nc.tensor.matmul(
                    ps,
                    lhsT=wq_sb[:, c, ec * P:(ec + 1) * P],
                    rhs=xT_sb[:, c, sl],
                    start=(c == 0), stop=(c == CC - 1),
                )
            # (x@W)*g + g*b fused into eviction, cast bf16
            nc.vector.tensor_scalar(
                QT[:, ec, sl], ps,
                gq_sb[:, ec:ec + 1], gbq_sb[:, ec:ec + 1],
                OP.mult, OP.add,
            )

        def kt_chain(ec, ss):
            sl = slice(ss * 512, (ss + 1) * 512)
            ps = projp.tile([P, 512], FP32, tag="proj", name=f"psk{ec}_{ss}")
            for c in range(CC):
                nc.tensor.matmul(
                    ps,
                    lhsT=wk_sb[:, c, ec * P:(ec + 1) * P],
                    rhs=xT_sb[:, c, sl],
                    start=(c == 0), stop=(c == CC - 1),
                )
            # per-head zero-padded to full 128 partitions so score matmuls
            # contract K=128 in the same (128,128) mode as the rest; the pad
            # halves are zeroed here per-chain (cheap bf16 DVE memsets that
            # overlap compute) instead of one giant up-front memset that
            # stalled the first evictions ~16us
            for hi in range(2):
                pp = slice(hi * Dh, (hi + 1) * Dh)
                po = slice((1 - hi) * Dh, (2 - hi) * Dh)
                nc.vector.tensor_scalar(
                    KTp[pp, 2 * ec + hi, sl], ps[pp, :],
                    gk_sb[pp, ec:ec + 1], gbk_sb[pp, ec:ec + 1],
                    OP.mult, OP.add,
                )
                nc.vector.memset(KTp[po, 2 * ec + hi, sl], 0.0)

        # ---- prefix: the minimum projection for the first exps ----
        kt_chain(0, 0)
        qt_chain(0, 0)
        qt_chain(0, 1)

        # remaining chains, interleaved into attention groups' PE slack.
        # group order: heads of one e-chunk first at sp0, then their sp1, so
        # each (KT ec, QT ec/sp) chain is needed as late as possible.
        # group 0 carries the V chains (V t-chunk t is only needed by its
        # lagged ctx matmul) plus KT ec0's remaining s-blocks.
        groups = [(0, 0), (1, 0), (0, 1), (1, 1),
                  (2, 0), (3, 0), (2, 1), (3, 1),
                  (4, 0), (5, 0), (4, 1), (5, 1),
                  (6, 0), (7, 0), (6, 1), (7, 1)]
        # per-group: {iter: [chains to emit after that iter's scores]}
        g0 = {}
        for t in range(8):
            g0[t] = [(lambda tt=2 * t: v_chain(tt)),
                     (lambda tt=2 * t + 1: v_chain(tt))]
        g0[1].append(lambda: kt_chain(0, 1))
        g0[3].append(lambda: kt_chain(0, 2))
        g0[5].append(lambda: kt_chain(0, 3))
        fillers = {
            0: g0,
            1: {2: [lambda: qt_chain(0, 2)], 8: [lambda: qt_chain(0, 3)]},
            2: {2: [lambda: kt_chain(1, 0)], 7: [lambda: kt_chain(1, 1)],
                12: [lambda: kt_chain(1, 2)]},
            3: {2: [lambda: kt_chain(1, 3)], 7: [lambda: qt_chain(1, 0)],
                12: [lambda: qt_chain(1, 1)]},
            4: {4: [lambda: qt_chain(1, 2)]},
            5: {4: [lambda: qt_chain(1, 3)], 10: [lambda: kt_chain(2, 0)]},
            6: {4: [lambda: kt_chain(2, 1)], 10: [lambda: kt_chain(2, 2)]},
            7: {2: [lambda: kt_chain(2, 3)], 7: [lambda: qt_chain(2, 0)],
                12: [lambda: qt_chain(2, 1)]},
            8: {4: [lambda: qt_chain(2, 2)]},
            9: {4: [lambda: qt_chain(2, 3)], 10: [lambda: kt_chain(3, 0)]},
            10: {4: [lambda: kt_chain(3, 1)], 10: [lambda: kt_chain(3, 2)]},
            11: {2: [lambda: kt_chain(3, 3)], 7: [lambda: qt_chain(3, 0)],
                 12: [lambda: qt_chain(3, 1)]},
            12: {4: [lambda: qt_chain(3, 2)]},
            13: {4: [lambda: qt_chain(3, 3)]},
        }

        # ---- attention groups ----
        # the per-group normalize is split: the DVE/DMA half (evict, pack,
        # reciprocal, spread) is emitted at group end and runs concurrently,
        # while the PE half (K=1 broadcast matmuls + multiply + store) is
        # deferred into the NEXT group's iteration stream so the PE never
        # stalls on the reciprocal dependency chain.
        pending_tail = []

        def emit_tail_pe(st):
            th, tsp, tcs, trcas = st
            thp, thi = th // 2, th % 2
            for j in range(2):
                sb = tsp * 2 + j
                rcb_ps = projp.tile([Dh, 512], FP32, tag="proj",
                                    name=f"rcb{th}_{tsp}_{j}")
                for q in range(2):
                    nc.tensor.matmul(
                        rcb_ps[:, q * 256:(q + 1) * 256],
                        lhsT=ones64[32 * q:32 * q + 1, :],
                        rhs=trcas[j][32 * q:32 * q + 1, :],
                        start=True, stop=True,
                    )
                ot = outp.tile([Dh, 512], FP32, tag="outsb")
                nc.vector.tensor_mul(ot, tcs[j][0:Dh, :], rcb_ps)
                nc.vector.tensor_scalar(
                    ot, ot,
                    bvT_sb[thi * Dh:(thi + 1) * Dh, thp:thp + 1], None,
                    OP.add,
                )
                nc.sync.dma_start(
                    out=out[th, :, sb * 512:(sb + 1) * 512], in_=ot)

        for g, (h, sp) in enumerate(groups):
            hp, hi = h // 2, h % 2
            fill = fillers.get(g, {})
            ctx_ps = [ctxp.tile([P, 512], FP32, tag="ctx",
                                name=f"ctx{g}_{k}") for k in range(2)]
            ets = {}
            for t in range(SC):
                sps = scp.tile([P, 1024], FP32, tag="sc")
                for j in range(2):
                    q0 = sp * 1024 + j * 512
                    nc.tensor.matmul(
                        sps[:, j * 512:(j + 1) * 512],
                        lhsT=KTp[:, h, t * P:(t + 1) * P],
                        rhs=QT[:, hp, q0:q0 + 512],
                        start=True, stop=True,
                    )
                et = etp.tile([P, 1024], BF16, tag="et")
                nc.scalar.activation(et, sps, AF.Exp)
                ets[t] = et
                # ctx lags scores by one t so PE always has queued work while
                # ACT exponentiates, and exp(t) is never gated on ctx
                if t > 0:
                    for j in range(2):
                        nc.tensor.matmul(
                            ctx_ps[j][0:Dh + 1, :],
                            lhsT=Vaug[:, t - 1, h, :],
                            rhs=ets[t - 1][:, j * 512:(j + 1) * 512],
                            start=(t - 1 == 0), stop=False,
                        )
                    del ets[t - 1]
                if t == 2 and pending_tail:
                    emit_tail_pe(pending_tail.pop(0))
                for chain in fill.get(t, ()):
                    chain()
            for j in range(2):
                nc.tensor.matmul(
                    ctx_ps[j][0:Dh + 1, :],
                    lhsT=Vaug[:, SC - 1, h, :],
                    rhs=ets[SC - 1][:, j * 512:(j + 1) * 512],
                    start=False, stop=True,
                )

            # ---- normalize + output ----
            # evict PSUM immediately; denominators (row 64 of each j block)
            # are packed onto 32-aligned partitions so (a) the reciprocal
            # runs multi-lane and (b) tiny K=1 ones-stationary matmuls
            # replicate 1/denom across 64 partitions straight into PSUM —
            # no high-latency broadcast DMAs on the tail's critical path.
            cs = []
            for j in range(2):
                c_t = rbp.tile([Dh + 1, 512], FP32, tag="cs")
                nc.vector.tensor_copy(c_t, ctx_ps[j][0:Dh + 1, :])
                cs.append(c_t)
            dpk = dkp.tile([8, P], FP32, tag="dpk")
            for j in range(2):
                nc.sync.dma_start(out=dpk[4 * j:4 * j + 4, :],
                                  in_=cs[j][Dh:Dh + 1, :])
            rct = dkp.tile([8, P], BF16, tag="rct")
            with nc.allow_low_precision(reason="1/denom cast bf16 for PE bcast"):
                nc.vector.reciprocal(rct, dpk)
            rcas = []
            for j in range(2):
                # spread 1/denom to partitions {0,32}; the PE broadcast +
                # multiply + store are deferred into the next group
                rca = dkp.tile([33, 256], BF16, tag="rca", name=f"rca{g}_{j}")
                nc.sync.dma_start(out=rca[0:1, :], in_=rct[4 * j:4 * j + 2, :])
                nc.sync.dma_start(out=rca[32:33, :], in_=rct[4 * j + 2:4 * j + 4, :])
                rcas.append(rca)
            pending_tail.append((h, sp, cs, rcas))

        while pending_tail:
            emit_tail_pe(pending_tail.pop(0))


def _build():
    key = "nc"
    if key in _CACHE:
        return _CACHE[key]
    nc = bacc.Bacc("TRN2", target_bir_lowering=False, debug=False,
                   enable_asserts=False)
    aps = {}

    def din(name, shape, dt):
        aps[name] = nc.dram_tensor(name, shape, dt, kind="ExternalInput").ap()

    din("xT", [P, CC, S], BF16)
    din("wq", [P, CC, E], BF16)
    din("wk", [P, CC, E], BF16)
    din("wv", [P, CC, E], BF16)
    din("wdq", [P, DC, E], BF16)
    din("wdk", [P, DC, E], BF16)
    din("txt", [T, DT], BF16)
    din("tmask", [T, 1], BF16)
    din("amask", [P, SC], FP32)
    din("bq", [P, ECH], FP32)
    din("bk", [P, ECH], FP32)
    din("bv", [P, ECH], FP32)
    din("bdq", [P, ECH], FP32)
    din("bdk", [P, ECH], FP32)
    aps["out"] = nc.dram_tensor("out", [HPC, Dh, S], FP32,
                                kind="ExternalOutput").ap()

    with tile.TileContext(nc) as tc:
        _emit(tc, aps)
    nc.compile()
    _CACHE[key] = nc
    return nc


def kernel(**inputs):
    global last_results
    hs = np.asarray(inputs["hidden_states"], dtype=np.float32)
    amask = np.asarray(inputs["attention_mask"], dtype=np.float32)
    txt = np.asarray(inputs["txt_embedding"], dtype=np.float32)
    tmask = np.asarray(inputs["txt_attention_mask"], dtype=np.float32)
    Wq = np.asarray(inputs["Wq"], dtype=np.float32)
    Wk = np.asarray(inputs["Wk"], dtype=np.float32)
    Wv = np.asarray(inputs["Wv"], dtype=np.float32)
    Wdq = np.asarray(inputs["Wdq"], dtype=np.float32)
    Wdk = np.asarray(inputs["Wdk"], dtype=np.float32)
    bq = np.asarray(inputs["bq"], dtype=np.float32)
    bk = np.asarray(inputs["bk"], dtype=np.float32)
    bv = np.asarray(inputs["bv"], dtype=np.float32)
    bdq = np.asarray(inputs["bdq"], dtype=np.float32)
    bdk = np.asarray(inputs["bdk"], dtype=np.float32)

    nc = _build()

    def pcs(w):
        # [D, e] -> [128, D//128, e]: partition-major so each partition's
        # DMA line is one contiguous block
        d = w.shape[0]
        return np.ascontiguousarray(
            w.reshape(d // P, P, w.shape[1]).transpose(1, 0, 2))

    def pvec(v):
        # [k*128] -> [128, k]
        return np.ascontiguousarray(v.reshape(-1, P).T)

    in_maps = []
    for c in range(NCORES):
        b, g = c // 2, c % 2
        cols = slice(g * E, (g + 1) * E)
        in_maps.append({
            "xT": pcs(np.ascontiguousarray(hs[b].T)).astype(BF16_NP),
            "wq": pcs(Wq[:, cols]).astype(BF16_NP),
            "wk": pcs(Wk[:, cols]).astype(BF16_NP),
            "wv": pcs(Wv[:, cols]).astype(BF16_NP),
            "wdq": pcs(Wdq[:, cols]).astype(BF16_NP),
            "wdk": pcs(Wdk[:, cols]).astype(BF16_NP),
            "txt": txt[b].astype(BF16_NP),
            "tmask": tmask[b].astype(BF16_NP),
            "amask": pvec(amask[b, 0, 0]),
            "bq": pvec(bq[cols]),
            "bk": pvec(bk[cols]),
            "bv": pvec(bv[cols]),
            "bdq": pvec(bdq[cols]),
            "bdk": pvec(bdk[cols]),
        })

    tr = int(os.environ.get("BASS_KERNEL_TRACE", "0"))
    if tr == 2:
        # warm the NEFF (compile+load+run untraced), then trace a second run
        run_bass_kernel_spmd(nc, in_maps, list(range(NCORES)), trace=False)
    res = run_bass_kernel_spmd(nc, in_maps, list(range(NCORES)), trace=bool(tr))
    last_results = res

    outp = np.empty((B, S, DV), dtype=np.float32)
    for c in range(NCORES):
        b, g = c // 2, c % 2
        # device output is ctx^T [head, d, s] -> [s, head*64+d]
        co = res.results[c]["out"].transpose(2, 0, 1).reshape(S, E)
        outp[b, :, g * E:(g + 1) * E] = co
    return outp


# revision 35
# speedup vs baseline: 1.0108x; 1.0073x over previous
"""BertImageSelfAttention Trainium2 kernel.

Shapes (fixed): hidden_states [4, 2048, 1024], 16 heads x 64, text [4, 64, 768].
Sharding: 8 cores = 4 batches x 2 head-groups (8 heads each). Each core computes
its batch's attention context for its 8 heads; host reassembles [4, 2048, 1024].

Per-core device pipeline (all matmuls bf16 with fp32 PSUM accumulation):
  A. pooled text -> dynamic Q/K gates (tiny matmuls + sigmoid)
  B. chunked loads: wk, wv, xT (4 s-blocks), wq on the SWDGE queue so the
     first KT projection chains start while the rest of xT streams in
  C. prefix projections: KT ec0 (interleaved with V per t-chunk as xT blocks
     land), QT ec0/sp0 -- just enough to start attention early; the remaining
     22 projection chains are interleaved into the attention groups' PE slack
     (deadline-scheduled so group g's inputs are always ready)
  D. 16 attention groups (h, sp): per t: S^T tile = K^T.T @ Q^T, ACT Exp
     (scale=1/8, bias=attention_mask), ctx^T[65,512] += Vaug.T @ E^T
     (ones column -> denominator in row 64); ctx lags scores by one t so the
     scalar engine (the bottleneck: 256 x ~1.3us Exp) never waits
  E. per group: evict ctx, pack the 4 denominator rows into one [8,128] tile,
     single full-lane reciprocal, broadcast via tiny DMAs, normalize + bias,
     DMA out [2048, 512] fp32 in ctx^T layout (host transposes).
"""

import os

import numpy as np
import ml_dtypes

import concourse.bass as bass
import concourse.bacc as bacc
import concourse.tile as tile
from concourse import mybir
from concourse.bass_utils import run_bass_kernel_spmd

P = 128
B, S, DV = 4, 2048, 1024
H, Dh = 16, 64
T, DT = 64, 768
NCORES = 8
E = 512          # head-group width (8 heads x 64)
CC = DV // P     # 8 contraction chunks for projections
ECH = E // P     # 4 e-chunks
DC = DT // P     # 6 text-dim chunks
SC = S // P      # 16 seq chunks of 128
SBL = S // 512   # 4 seq blocks of 512
HPC = 8          # heads per core

FP32 = mybir.dt.float32
BF16 = mybir.dt.bfloat16
AF = mybir.ActivationFunctionType
OP = mybir.AluOpType

BF16_NP = ml_dtypes.bfloat16

_CACHE = {}

# module-level stash of the last BassKernelResults (for test.py introspection)
last_results = None


def _emit(tc, aps):
    nc = tc.nc
    # all inputs arrive pre-permuted from the host so every DMA reads
    # contiguous per-partition lines (the lowered "(c p) -> p c" scatter
    # patterns measured 4-40x slower than contiguous loads)
    xT = aps["xT"]                                             # [128, 8, 2048]
    wq = aps["wq"]                                             # [128, 8, 512]
    wk = aps["wk"]
    wv = aps["wv"]
    wdq = aps["wdq"]                                           # [128, 6, 512]
    wdk = aps["wdk"]
    txt = aps["txt"]                                           # [64, 768] bf16
    tmask = aps["tmask"]                                       # [64, 1] bf16
    amask = aps["amask"]                                       # [128, 16]
    bq = aps["bq"]                                             # [128, 4]
    bk = aps["bk"]
    bdq = aps["bdq"]
    bdk = aps["bdk"]
    bv = aps["bv"]                                             # [128, 4] (bvT)
    out = aps["out"]                                           # [8, 64, 2048] f32

    from contextlib import ExitStack

    with ExitStack() as ctx:
        wpool = ctx.enter_context(tc.tile_pool(name="wpool", bufs=1))
        xpool = ctx.enter_context(tc.tile_pool(name="xpool", bufs=1))
        qkpool = ctx.enter_context(tc.tile_pool(name="qkpool", bufs=1))
        vpool = ctx.enter_context(tc.tile_pool(name="vpool", bufs=1))
        etp = ctx.enter_context(tc.tile_pool(name="etp", bufs=8))
        rbp = ctx.enter_context(tc.tile_pool(name="rbp", bufs=5))
        outp = ctx.enter_context(tc.tile_pool(name="outp", bufs=4))
        smallp = ctx.enter_context(tc.tile_pool(name="smallp", bufs=1))
        dkp = ctx.enter_context(tc.tile_pool(name="dkp", bufs=8))
        # PSUM: 8 banks = scp 2 x [128,1024] (4) + ctxp 2 x [128,512] (2)
        # + projp 2 x [128,512] (2).  projp hosts the interleaved projection
        # chains (and the gate matmuls in the prefix).
        ctxp = ctx.enter_context(tc.tile_pool(name="ctxp", bufs=2, space="PSUM"))
        projp = ctx.enter_context(tc.tile_pool(name="projp", bufs=2, space="PSUM"))
        scp = ctx.enter_context(tc.tile_pool(name="scp", bufs=2, space="PSUM"))

        # text tensors padded to 128 partitions (zero rows 64..127) so every
        # matmul runs in uniform (128,128) PE tile mode — no mode switches.
        txt_sb = smallp.tile([P, DT], BF16, tag="txt")
        nc.vector.memset(txt_sb[T:P, :], 0.0)
        nc.sync.dma_start(out=txt_sb[0:T, :], in_=txt)
        # mask as a [128,128] stationary: column 0 = mask, rest zero -> M=128
        tmask_sb = smallp.tile([P, P], BF16, tag="tmask")
        nc.vector.memset(tmask_sb, 0.0)
        nc.sync.dma_start(out=tmask_sb[0:T, 0:1], in_=tmask)
        ones_sb = smallp.tile([P, 1], BF16, tag="ones")
        nc.vector.memset(ones_sb, 1.0)
        amask_sb = smallp.tile([P, SC], FP32, tag="amask")
        nc.sync.dma_start(out=amask_sb, in_=amask)
        bq_sb = smallp.tile([P, ECH], FP32, tag="bq")
        nc.sync.dma_start(out=bq_sb, in_=bq)
        bk_sb = smallp.tile([P, ECH], FP32, tag="bk")
        nc.sync.dma_start(out=bk_sb, in_=bk)
        bdq_sb = smallp.tile([P, ECH], FP32, tag="bdq")
        nc.sync.dma_start(out=bdq_sb, in_=bdq)
        bdk_sb = smallp.tile([P, ECH], FP32, tag="bdk")
        nc.sync.dma_start(out=bdk_sb, in_=bdk)
        bvT_sb = smallp.tile([P, ECH], FP32, tag="bvT")
        nc.sync.dma_start(out=bvT_sb, in_=bv)

        # ---- big loads, split across all three DMA queues so the critical
        # pieces land in parallel: SWDGE carries KT/QT ec0 inputs (wk, xT
        # blocks 0-1, wq), the scalar HWDGE queue carries wv + xT blocks 2-3,
        # and the sync queue (after the smalls) carries the gate weights ----
        wdq_sb = wpool.tile([P, DC, E], BF16, tag="wdq")
        nc.sync.dma_start(out=wdq_sb, in_=wdq)
        wdk_sb = wpool.tile([P, DC, E], BF16, tag="wdk")
        nc.sync.dma_start(out=wdk_sb, in_=wdk)
        wk_sb = wpool.tile([P, CC, E], BF16, tag="wk")
        nc.gpsimd.dma_start(out=wk_sb, in_=wk)
        xT_sb = xpool.tile([P, CC, S], BF16, tag="xT")
        nc.gpsimd.dma_start(out=xT_sb[:, :, 0:512], in_=xT[:, :, 0:512])
        wq_sb = wpool.tile([P, CC, E], BF16, tag="wq")
        nc.gpsimd.dma_start(out=wq_sb, in_=wq)
        nc.gpsimd.dma_start(out=xT_sb[:, :, 512:1024], in_=xT[:, :, 512:1024])
        wv_sb = wpool.tile([P, CC, E], BF16, tag="wv")
        nc.scalar.dma_start(out=wv_sb, in_=wv)
        nc.scalar.dma_start(out=xT_sb[:, :, 1024:1536], in_=xT[:, :, 1024:1536])
        nc.scalar.dma_start(out=xT_sb[:, :, 1536:2048], in_=xT[:, :, 1536:2048])

        # ---- gates: pooled text + sigmoid ----
        pr = scp.tile([P, 769], FP32, tag="sc")
        nc.tensor.matmul(pr[:, 0:512], lhsT=tmask_sb, rhs=txt_sb[:, 0:512],
                         start=True, stop=True)
        nc.tensor.matmul(pr[:, 512:768], lhsT=tmask_sb, rhs=txt_sb[:, 512:768],
                         start=True, stop=True)
        nc.tensor.matmul(pr[:, 768:769], lhsT=tmask_sb, rhs=ones_sb,
                         start=True, stop=True)
        rmsum = smallp.tile([1, 1], FP32, tag="rmsum")
        nc.vector.reciprocal(rmsum, pr[0:1, 768:769])
        prow = smallp.tile([1, DT], BF16, tag="prow")
        nc.vector.tensor_scalar(prow, pr[0:1, 0:768], rmsum, None, OP.mult)

        # scatter pooled row -> poolT [128, 6] (dt on partitions) via tiny
        # SBUF->SBUF DMA (dt = c*128 + p), spread over three DMA queues
        poolT = smallp.tile([P, DC], BF16, tag="poolT")
        for c in range(DC):
            dq = (nc.sync, nc.scalar)[c % 2]
            dq.dma_start(
                out=poolT[:, c:c + 1],
                in_=prow[0:1, c * P:(c + 1) * P],
            )

        # gates: g = 1 + sigmoid(pool @ Wd + bd); also g*b for fused bias
        gq_sb = smallp.tile([P, ECH], FP32, tag="gq")
        gk_sb = smallp.tile([P, ECH], FP32, tag="gk")
        gbq_sb = smallp.tile([P, ECH], FP32, tag="gbq")
        gbk_sb = smallp.tile([P, ECH], FP32, tag="gbk")
        # the softmax 1/sqrt(Dh)=0.125 scale is folded into the Q gate so the
        # Exp activation needs no scale operand: g_q = 0.125*(1+sigmoid(...))
        for (wd_sb, bd_sb, b_sb, g_sb, gb_sb, gsc) in (
            (wdq_sb, bdq_sb, bq_sb, gq_sb, gbq_sb, 0.125),
            (wdk_sb, bdk_sb, bk_sb, gk_sb, gbk_sb, 1.0),
        ):
            for ec in range(ECH):
                gp = projp.tile([P, 512], FP32, tag="proj")
                for c in range(DC):
                    nc.tensor.matmul(
                        gp[:, 0:1],
                        lhsT=wd_sb[:, c, ec * P:(ec + 1) * P],
                        rhs=poolT[:, c:c + 1],
                        start=(c == 0), stop=(c == DC - 1),
                    )
                nc.scalar.activation(g_sb[:, ec:ec + 1], gp[:, 0:1], AF.Sigmoid,
                                     bias=bd_sb[:, ec:ec + 1])
            nc.vector.tensor_scalar(g_sb, g_sb, gsc, gsc, OP.mult, OP.add)
            nc.vector.tensor_mul(gb_sb, g_sb, b_sb)

        # exp(mask) folded into V and the denominator column:
        # exp(s/8 + m[k]) = exp(s/8) * em[k], and em scales per KEY = per
        # partition of Vaug, so the Exp activation needs no bias operand.
        em_sb = smallp.tile([P, SC], FP32, tag="em")
        nc.scalar.activation(em_sb, amask_sb, AF.Exp)
        ones8 = smallp.tile([P, HPC, 1], BF16, tag="ones8")
        nc.vector.memset(ones8, 1.0)
        ones64 = smallp.tile([P, Dh], BF16, tag="ones64")
        nc.vector.memset(ones64, 1.0)

        # ---- persistent SBUF tensors for projections ----
        Vaug = vpool.tile([P, SC, HPC, Dh + 1], BF16, tag="Vaug")
        QT = qkpool.tile([P, ECH, S], BF16, tag="QT")
        KTp = qkpool.tile([P, HPC, S], BF16, tag="KTp")

        # ---- projection chain emitters (8 accumulation MMs + eviction) ----
        def v_chain(t):
            ps = projp.tile([P, 512], FP32, tag="proj", name=f"psv{t}")
            for c in range(CC):
                nc.tensor.matmul(
                    ps,
                    lhsT=xT_sb[:, c, t * P:(t + 1) * P],
                    rhs=wv_sb[:, c, :],
                    start=(c == 0), stop=(c == CC - 1),
                )
            nc.vector.tensor_scalar(
                Vaug[:, t, :, 0:Dh],
                ps.rearrange("p (h d) -> p h d", h=HPC),
                em_sb[:, t:t + 1], None, OP.mult,
            )
            nc.vector.tensor_scalar(
                Vaug[:, t, :, Dh:Dh + 1], ones8,
                em_sb[:, t:t + 1], None, OP.mult,
            )

        def qt_chain(ec, ss):
            sl = slice(ss * 512, (ss + 1) * 512)
            ps = projp.tile([P, 512], FP32, tag="proj", name=f"psq{ec}_{ss}")
            for c in range(CC):
                nc.tensor.matmul(
                    ps,
                    lhsT=wq_sb[:, c, ec * P:(ec + 1) * P],
                    rhs=xT_sb[:, c, sl],
                    start=(c == 0), stop=(c == CC - 1),
                )
            # (x@W)*g + g*b fused into eviction, cast bf16
            nc.vector.tensor_scalar(
                QT[:, ec, sl], ps,
                gq_sb[:, ec:ec + 1], gbq_sb[:, ec:ec + 1],
                OP.mult, OP.add,
            )

        def kt_chain(ec, ss):
            sl = slice(ss * 512, (ss + 1) * 512)
            ps = projp.tile([P, 512], FP32, tag="proj", name=f"psk{ec}_{ss}")
            for c in range(CC):
                nc.tensor.matmul(
                    ps,
                    lhsT=wk_sb[:, c, ec * P:(ec + 1) * P],
                    rhs=xT_sb[:, c, sl],
                    start=(c == 0), stop=(c == CC - 1),
                )
            # per-head zero-padded to full 128 partitions so score matmuls
            # contract K=128 in the same (128,128) mode as the rest; the pad
            # halves are zeroed here per-chain (cheap bf16 DVE memsets that
            # overlap compute) instead of one giant up-front memset that
            # stalled the first evictions ~16us
            for hi in range(2):
                pp = slice(hi * Dh, (hi + 1) * Dh)
                po = slice((1 - hi) * Dh, (2 - hi) * Dh)
                nc.vector.tensor_scalar(
                    KTp[pp, 2 * ec + hi, sl], ps[pp, :],
                    gk_sb[pp, ec:ec + 1], gbk_sb[pp, ec:ec + 1],
                    OP.mult, OP.add,
                )
                nc.vector.memset(KTp[po, 2 * ec + hi, sl], 0.0)

        # ---- prefix: the minimum projection for the first exps ----
        kt_chain(0, 0)
        qt_chain(0, 0)
        qt_chain(0, 1)

        # remaining chains, interleaved into attention groups' PE slack.
        # group order: heads of one e-chunk first at sp0, then their sp1, so
        # each (KT ec, QT ec/sp) chain is needed as late as possible.
        # group 0 carries the V chains (V t-chunk t is only needed by its
        # lagged ctx matmul) plus KT ec0's remaining s-blocks.
        groups = [(0, 0), (1, 0), (0, 1), (1, 1),
                  (2, 0), (3, 0), (2, 1), (3, 1),
                  (4, 0), (5, 0), (4, 1), (5, 1),
                  (6, 0), (7, 0), (6, 1), (7, 1)]
        # per-group: {iter: [chains to emit after that iter's scores]}
        g0 = {}
        for t in range(8):
            g0[t] = [(lambda tt=2 * t: v_chain(tt)),
                     (lambda tt=2 * t + 1: v_chain(tt))]
        g0[1].append(lambda: kt_chain(0, 1))
        g0[3].append(lambda: kt_chain(0, 2))
        g0[5].append(lambda: kt_chain(0, 3))
        fillers = {
            0: g0,
            1: {2: [lambda: qt_chain(0, 2)], 8: [lambda: qt_chain(0, 3)]},
            2: {2: [lambda: kt_chain(1, 0)], 7: [lambda: kt_chain(1, 1)],
                12: [lambda: kt_chain(1, 2)]},
            3: {2: [lambda: kt_chain(1, 3)], 7: [lambda: qt_chain(1, 0)],
                12: [lambda: qt_chain(1, 1)]},
            4: {4: [lambda: qt_chain(1, 2)]},
            5: {4: [lambda: qt_chain(1, 3)], 10: [lambda: kt_chain(2, 0)]},
            6: {4: [lambda: kt_chain(2, 1)], 10: [lambda: kt_chain(2, 2)]},
            7: {2: [lambda: kt_chain(2, 3)], 7: [lambda: qt_chain(2, 0)],
                12: [lambda: qt_chain(2, 1)]},
            8: {4: [lambda: qt_chain(2, 2)]},
            9: {4: [lambda: qt_chain(2, 3)], 10: [lambda: kt_chain(3, 0)]},
            10: {4: [lambda: kt_chain(3, 1)], 10: [lambda: kt_chain(3, 2)]},
            11: {2: [lambda: kt_chain(3, 3)], 7: [lambda: qt_chain(3, 0)],
                 12: [lambda: qt_chain(3, 1)]},
            12: {4: [lambda: qt_chain(3, 2)]},
            13: {4: [lambda: qt_chain(3, 3)]},
        }

        # ---- attention groups ----
        # the per-group normalize is split: the DVE/DMA half (evict, pack,
        # reciprocal, spread) is emitted at group end and runs concurrently,
        # while the PE half (K=1 broadcast matmuls + multiply + store) is
        # deferred into the NEXT group's iteration stream so the PE never
        # stalls on the reciprocal dependency chain.
        pending_tail = []

        def emit_tail_pe(st):
            th, tsp, tcs, trcas = st
            thp, thi = th // 2, th % 2
            for j in range(2):
                sb = tsp * 2 + j
                rcb_ps = projp.tile([Dh, 512], FP32, tag="proj",
                                    name=f"rcb{th}_{tsp}_{j}")
                for q in range(2):
                    nc.tensor.matmul(
                        rcb_ps[:, q * 256:(q + 1) * 256],
                        lhsT=ones64[32 * q:32 * q + 1, :],
                        rhs=trcas[j][32 * q:32 * q + 1, :],
                        start=True, stop=True,
                    )
                ot = outp.tile([Dh, 512], FP32, tag="outsb")
                nc.vector.tensor_mul(ot, tcs[j][0:Dh, :], rcb_ps)
                nc.vector.tensor_scalar(
                    ot, ot,
                    bvT_sb[thi * Dh:(thi + 1) * Dh, thp:thp + 1], None,
                    OP.add,
                )
                nc.sync.dma_start(
                    out=out[th, :, sb * 512:(sb + 1) * 512], in_=ot)

        for g, (h, sp) in enumerate(groups):
            hp, hi = h // 2, h % 2
            fill = fillers.get(g, {})
            ctx_ps = [ctxp.tile([P, 512], FP32, tag="ctx",
                                name=f"ctx{g}_{k}") for k in range(2)]
            ets = {}
            for t in range(SC):
                sps = scp.tile([P, 1024], FP32, tag="sc")
                for j in range(2):
                    q0 = sp * 1024 + j * 512
                    nc.tensor.matmul(
                        sps[:, j * 512:(j + 1) * 512],
                        lhsT=KTp[:, h, t * P:(t + 1) * P],
                        rhs=QT[:, hp, q0:q0 + 512],
                        start=True, stop=True,
                    )
                et = etp.tile([P, 1024], BF16, tag="et")
                nc.scalar.activation(et, sps, AF.Exp)
                ets[t] = et
                # ctx lags scores by one t so PE always has queued work while
                # ACT exponentiates, and exp(t) is never gated on ctx
                if t > 0:
                    for j in range(2):
                        nc.tensor.matmul(
                            ctx_ps[j][0:Dh + 1, :],
                            lhsT=Vaug[:, t - 1, h, :],
                            rhs=ets[t - 1][:, j * 512:(j + 1) * 512],
                            start=(t - 1 == 0), stop=False,
                        )
                    del ets[t - 1]
                if t == 2 and pending_tail:
                    emit_tail_pe(pending_tail.pop(0))
                for chain in fill.get(t, ()):
                    chain()
            for j in range(2):
                nc.tensor.matmul(
                    ctx_ps[j][0:Dh + 1, :],
                    lhsT=Vaug[:, SC - 1, h, :],
                    rhs=ets[SC - 1][:, j * 512:(j + 1) * 512],
                    start=False, stop=True,
                )

            # ---- normalize + output ----
            # evict PSUM immediately; denominators (row 64 of each j block)
            # are packed onto 32-aligned partitions so (a) the reciprocal
            # runs multi-lane and (b) tiny K=1 ones-stationary matmuls
            # replicate 1/denom across 64 partitions straight into PSUM —
            # no high-latency broadcast DMAs on the tail's critical path.
            cs = []
            for j in range(2):
                c_t = rbp.tile([Dh + 1, 512], FP32, tag="cs")
                nc.vector.tensor_copy(c_t, ctx_ps[j][0:Dh + 1, :])
                cs.append(c_t)
            dpk = dkp.tile([8, P], FP32, tag="dpk")
            for j in range(2):
                nc.sync.dma_start(out=dpk[4 * j:4 * j + 4, :],
                                  in_=cs[j][Dh:Dh + 1, :])
            rct = dkp.tile([8, P], BF16, tag="rct")
            with nc.allow_low_precision(reason="1/denom cast bf16 for PE bcast"):
                nc.vector.reciprocal(rct, dpk)
            rcas = []
            for j in range(2):
                # spread 1/denom to partitions {0,32}; the PE broadcast +
                # multiply + store are deferred into the next group
                rca = dkp.tile([33, 256], BF16, tag="rca", name=f"rca{g}_{j}")
                nc.sync.dma_start(out=rca[0:1, :], in_=rct[4 * j:4 * j + 2, :])
                nc.sync.dma_start(out=rca[32:33, :], in_=rct[4 * j + 2:4 * j + 4, :])
                rcas.append(rca)
            pending_tail.append((h, sp, cs, rcas))

        while pending_tail:
            emit_tail_pe(pending_tail.pop(0))


def _build():
    key = "nc"
    if key in _CACHE:
        return _CACHE[key]
    nc = bacc.Bacc("TRN2", target_bir_lowering=False, debug=False,
                   enable_asserts=False)
    aps = {}

    def din(name, shape, dt):
        aps[name] = nc.dram_tensor(name, shape, dt, kind="ExternalInput").ap()

    din("xT", [P, CC, S], BF16)
    din("wq", [P, CC, E], BF16)
    din("wk", [P, CC, E], BF16)
    din("wv", [P, CC, E], BF16)
    din("wdq", [P, DC, E], BF16)
    din("wdk", [P, DC, E], BF16)
    din("txt", [T, DT], BF16)
    din("tmask", [T, 1], BF16)
    din("amask", [P, SC], FP32)
    din("bq", [P, ECH], FP32)
    din("bk", [P, ECH], FP32)
    din("bv", [P, ECH], FP32)
    din("bdq", [P, ECH], FP32)
    din("bdk", [P, ECH], FP32)
    aps["out"] = nc.dram_tensor("out", [HPC, Dh, S], FP32,
                                kind="ExternalOutput").ap()

    with tile.TileContext(nc) as tc:
        _emit(tc, aps)
    nc.compile()
    _CACHE[key] = nc
    return nc


def kernel(**inputs):
    global last_results
    hs = np.asarray(inputs["hidden_states"], dtype=np.float32)
    amask = np.asarray(inputs["attention_mask"], dtype=np.float32)
    txt = np.asarray(inputs["txt_embedding"], dtype=np.float32)
    tmask = np.asarray(inputs["txt_attention_mask"], dtype=np.float32)
    Wq = np.asarray(inputs["Wq"], dtype=np.float32)
    Wk = np.asarray(inputs["Wk"], dtype=np.float32)
    Wv = np.asarray(inputs["Wv"], dtype=np.float32)
    Wdq = np.asarray(inputs["Wdq"], dtype=np.float32)
    Wdk = np.asarray(inputs["Wdk"], dtype=np.float32)
    bq = np.asarray(inputs["bq"], dtype=np.float32)
    bk = np.asarray(inputs["bk"], dtype=np.float32)
    bv = np.asarray(inputs["bv"], dtype=np.float32)
    bdq = np.asarray(inputs["bdq"], dtype=np.float32)
    bdk = np.asarray(inputs["bdk"], dtype=np.float32)

    nc = _build()

    def pcs(w):
        # [D, e] -> [128, D//128, e]: partition-major so each partition's
        # DMA line is one contiguous block
        d = w.shape[0]
        return np.ascontiguousarray(
            w.reshape(d // P, P, w.shape[1]).transpose(1, 0, 2))

    def pvec(v):
        # [k*128] -> [128, k]
        return np.ascontiguousarray(v.reshape(-1, P).T)

    in_maps = []
    for c in range(NCORES):
        b, g = c // 2, c % 2
        cols = slice(g * E, (g + 1) * E)
        in_maps.append({
            "xT": pcs(np.ascontiguousarray(hs[b].T)).astype(BF16_NP),
            "wq": pcs(Wq[:, cols]).astype(BF16_NP),
            "wk": pcs(Wk[:, cols]).astype(BF16_NP),
            "wv": pcs(Wv[:, cols]).astype(BF16_NP),
            "wdq": pcs(Wdq[:, cols]).astype(BF16_NP),
            "wdk": pcs(Wdk[:, cols]).astype(BF16_NP),
            "txt": txt[b].astype(BF16_NP),
            "tmask": tmask[b].astype(BF16_NP),
            "amask": pvec(amask[b, 0, 0]),
            "bq": pvec(bq[cols]),
            "bk": pvec(bk[cols]),
            "bv": pvec(bv[cols]),
            "bdq": pvec(bdq[cols]),
            "bdk": pvec(bdk[cols]),
        })

    tr = int(os.environ.get("BASS_KERNEL_TRACE", "0"))
    if tr == 2:
        # warm the NEFF (compile+load+run untraced), then trace a second run
        run_bass_kernel_spmd(nc, in_maps, list(range(NCORES)), trace=False)
    res = run_bass_kernel_spmd(nc, in_maps, list(range(NCORES)), trace=bool(tr))
    last_results = res

    outp = np.empty((B, S, DV), dtype=np.float32)
    for c in range(NCORES):
        b, g = c // 2, c % 2
        # device output is ctx^T [head, d, s] -> [s, head*64+d]
        co = res.results[c]["out"].transpose(2, 0, 1).reshape(S, E)
        outp[b, :, g * E:(g + 1) * E] = co
    return outp


# revision 36
# speedup vs baseline: 1.1109x; 1.0990x over previous
"""BertImageSelfAttention Trainium2 kernel.

Shapes (fixed): hidden_states [4, 2048, 1024], 16 heads x 64, text [4, 64, 768].
Sharding: 8 cores = 4 batches x 2 head-groups (8 heads each). Each core computes
its batch's attention context for its 8 heads; host reassembles [4, 2048, 1024].

Per-core device pipeline (all matmuls bf16 with fp32 PSUM accumulation):
  A. pooled text -> dynamic Q/K gates (tiny matmuls + sigmoid)
  B. chunked loads: wk, wv, xT (4 s-blocks), wq on the SWDGE queue so the
     first KT projection chains start while the rest of xT streams in
  C. prefix projections: KT ec0 (interleaved with V per t-chunk as xT blocks
     land), QT ec0/sp0 -- just enough to start attention early; the remaining
     22 projection chains are interleaved into the attention groups' PE slack
     (deadline-scheduled so group g's inputs are always ready)
  D. 16 attention groups (h, sp): per t: S^T tile = K^T.T @ Q^T, ACT Exp
     (scale=1/8, bias=attention_mask), ctx^T[65,512] += Vaug.T @ E^T
     (ones column -> denominator in row 64); ctx lags scores by one t so the
     scalar engine (the bottleneck: 256 x ~1.3us Exp) never waits
  E. per group: evict ctx, pack the 4 denominator rows into one [8,128] tile,
     single full-lane reciprocal, broadcast via tiny DMAs, normalize + bias,
     DMA out [2048, 512] fp32 in ctx^T layout (host transposes).
"""

import os

import numpy as np
import ml_dtypes

import concourse.bass as bass
import concourse.bacc as bacc
import concourse.tile as tile
from concourse import mybir
from concourse.bass_utils import run_bass_kernel_spmd

P = 128
B, S, DV = 4, 2048, 1024
H, Dh = 16, 64
T, DT = 64, 768
NCORES = 8
E = 512          # head-group width (8 heads x 64)
CC = DV // P     # 8 contraction chunks for projections
ECH = E // P     # 4 e-chunks
DC = DT // P     # 6 text-dim chunks
SC = S // P      # 16 seq chunks of 128
SBL = S // 512   # 4 seq blocks of 512
HPC = 8          # heads per core

FP32 = mybir.dt.float32
BF16 = mybir.dt.bfloat16
AF = mybir.ActivationFunctionType
OP = mybir.AluOpType

BF16_NP = ml_dtypes.bfloat16

_CACHE = {}

# module-level stash of the last BassKernelResults (for test.py introspection)
last_results = None


def _emit(tc, aps):
    nc = tc.nc
    # all inputs arrive pre-permuted from the host so every DMA reads
    # contiguous per-partition lines (the lowered "(c p) -> p c" scatter
    # patterns measured 4-40x slower than contiguous loads)
    xT = aps["xT"]                                             # [128, 8, 2048]
    wq = aps["wq"]                                             # [128, 8, 512]
    wk = aps["wk"]
    wv = aps["wv"]
    wdq = aps["wdq"]                                           # [128, 6, 512]
    wdk = aps["wdk"]
    txt = aps["txt"]                                           # [64, 768] bf16
    tmask = aps["tmask"]                                       # [64, 1] bf16
    amask = aps["amask"]                                       # [128, 16]
    bq = aps["bq"]                                             # [128, 4]
    bk = aps["bk"]
    bdq = aps["bdq"]
    bdk = aps["bdk"]
    bv = aps["bv"]                                             # [128, 4] (bvT)
    out = aps["out"]                                           # [8, 64, 2048] f32

    from contextlib import ExitStack

    with ExitStack() as ctx:
        wpool = ctx.enter_context(tc.tile_pool(name="wpool", bufs=1))
        xpool = ctx.enter_context(tc.tile_pool(name="xpool", bufs=1))
        qkpool = ctx.enter_context(tc.tile_pool(name="qkpool", bufs=1))
        vpool = ctx.enter_context(tc.tile_pool(name="vpool", bufs=1))
        etp = ctx.enter_context(tc.tile_pool(name="etp", bufs=8))
        rbp = ctx.enter_context(tc.tile_pool(name="rbp", bufs=6))
        outp = ctx.enter_context(tc.tile_pool(name="outp", bufs=4))
        smallp = ctx.enter_context(tc.tile_pool(name="smallp", bufs=1))
        dkp = ctx.enter_context(tc.tile_pool(name="dkp", bufs=8))
        # PSUM: 8 banks = scp 2 x [128,1024] (4) + ctxp 2 x [128,512] (2)
        # + projp 2 x [128,512] (2).  projp hosts the interleaved projection
        # chains (and the gate matmuls in the prefix).
        ctxp = ctx.enter_context(tc.tile_pool(name="ctxp", bufs=2, space="PSUM"))
        projp = ctx.enter_context(tc.tile_pool(name="projp", bufs=2, space="PSUM"))
        scp = ctx.enter_context(tc.tile_pool(name="scp", bufs=2, space="PSUM"))

        # text tensors padded to 128 partitions (zero rows 64..127) so every
        # matmul runs in uniform (128,128) PE tile mode — no mode switches.
        txt_sb = smallp.tile([P, DT], BF16, tag="txt")
        nc.vector.memset(txt_sb[T:P, :], 0.0)
        nc.sync.dma_start(out=txt_sb[0:T, :], in_=txt)
        # mask as a [128,128] stationary: column 0 = mask, rest zero -> M=128
        tmask_sb = smallp.tile([P, P], BF16, tag="tmask")
        nc.vector.memset(tmask_sb, 0.0)
        nc.sync.dma_start(out=tmask_sb[0:T, 0:1], in_=tmask)
        ones_sb = smallp.tile([P, 1], BF16, tag="ones")
        nc.vector.memset(ones_sb, 1.0)
        amask_sb = smallp.tile([P, SC], FP32, tag="amask")
        nc.sync.dma_start(out=amask_sb, in_=amask)
        bq_sb = smallp.tile([P, ECH], FP32, tag="bq")
        nc.sync.dma_start(out=bq_sb, in_=bq)
        bk_sb = smallp.tile([P, ECH], FP32, tag="bk")
        nc.sync.dma_start(out=bk_sb, in_=bk)
        bdq_sb = smallp.tile([P, ECH], FP32, tag="bdq")
        nc.sync.dma_start(out=bdq_sb, in_=bdq)
        bdk_sb = smallp.tile([P, ECH], FP32, tag="bdk")
        nc.sync.dma_start(out=bdk_sb, in_=bdk)
        bvT_sb = smallp.tile([P, ECH], FP32, tag="bvT")
        nc.sync.dma_start(out=bvT_sb, in_=bv)

        # ---- big loads, split across all three DMA queues so the critical
        # pieces land in parallel: SWDGE carries KT/QT ec0 inputs (wk, xT
        # blocks 0-1, wq), the scalar HWDGE queue carries wv + xT blocks 2-3,
        # and the sync queue (after the smalls) carries the gate weights ----
        wdq_sb = wpool.tile([P, DC, E], BF16, tag="wdq")
        nc.sync.dma_start(out=wdq_sb, in_=wdq)
        wdk_sb = wpool.tile([P, DC, E], BF16, tag="wdk")
        nc.sync.dma_start(out=wdk_sb, in_=wdk)
        wk_sb = wpool.tile([P, CC, E], BF16, tag="wk")
        nc.gpsimd.dma_start(out=wk_sb, in_=wk)
        xT_sb = xpool.tile([P, CC, S], BF16, tag="xT")
        nc.gpsimd.dma_start(out=xT_sb[:, :, 0:512], in_=xT[:, :, 0:512])
        wq_sb = wpool.tile([P, CC, E], BF16, tag="wq")
        nc.gpsimd.dma_start(out=wq_sb, in_=wq)
        nc.gpsimd.dma_start(out=xT_sb[:, :, 512:1024], in_=xT[:, :, 512:1024])
        wv_sb = wpool.tile([P, CC, E], BF16, tag="wv")
        nc.scalar.dma_start(out=wv_sb, in_=wv)
        nc.scalar.dma_start(out=xT_sb[:, :, 1024:1536], in_=xT[:, :, 1024:1536])
        nc.scalar.dma_start(out=xT_sb[:, :, 1536:2048], in_=xT[:, :, 1536:2048])

        # ---- gates: pooled text + sigmoid ----
        pr = scp.tile([P, 769], FP32, tag="sc")
        nc.tensor.matmul(pr[:, 0:512], lhsT=tmask_sb, rhs=txt_sb[:, 0:512],
                         start=True, stop=True)
        nc.tensor.matmul(pr[:, 512:768], lhsT=tmask_sb, rhs=txt_sb[:, 512:768],
                         start=True, stop=True)
        nc.tensor.matmul(pr[:, 768:769], lhsT=tmask_sb, rhs=ones_sb,
                         start=True, stop=True)
        rmsum = smallp.tile([1, 1], FP32, tag="rmsum")
        nc.vector.reciprocal(rmsum, pr[0:1, 768:769])
        prow = smallp.tile([1, DT], BF16, tag="prow")
        nc.vector.tensor_scalar(prow, pr[0:1, 0:768], rmsum, None, OP.mult)

        # scatter pooled row -> poolT [128, 6] (dt on partitions) via tiny
        # SBUF->SBUF DMA (dt = c*128 + p), spread over three DMA queues
        poolT = smallp.tile([P, DC], BF16, tag="poolT")
        for c in range(DC):
            dq = (nc.sync, nc.scalar)[c % 2]
            dq.dma_start(
                out=poolT[:, c:c + 1],
                in_=prow[0:1, c * P:(c + 1) * P],
            )

        # gates: g = 1 + sigmoid(pool @ Wd + bd); also g*b for fused bias
        gq_sb = smallp.tile([P, ECH], FP32, tag="gq")
        gk_sb = smallp.tile([P, ECH], FP32, tag="gk")
        gbq_sb = smallp.tile([P, ECH], FP32, tag="gbq")
        gbk_sb = smallp.tile([P, ECH], FP32, tag="gbk")
        # the softmax 1/sqrt(Dh)=0.125 scale is folded into the Q gate so the
        # Exp activation needs no scale operand: g_q = 0.125*(1+sigmoid(...))
        for (wd_sb, bd_sb, b_sb, g_sb, gb_sb, gsc) in (
            (wdq_sb, bdq_sb, bq_sb, gq_sb, gbq_sb, 0.125),
            (wdk_sb, bdk_sb, bk_sb, gk_sb, gbk_sb, 1.0),
        ):
            for ec in range(ECH):
                gp = projp.tile([P, 512], FP32, tag="proj")
                for c in range(DC):
                    nc.tensor.matmul(
                        gp[:, 0:1],
                        lhsT=wd_sb[:, c, ec * P:(ec + 1) * P],
                        rhs=poolT[:, c:c + 1],
                        start=(c == 0), stop=(c == DC - 1),
                    )
                nc.scalar.activation(g_sb[:, ec:ec + 1], gp[:, 0:1], AF.Sigmoid,
                                     bias=bd_sb[:, ec:ec + 1])
            nc.vector.tensor_scalar(g_sb, g_sb, gsc, gsc, OP.mult, OP.add)
            nc.vector.tensor_mul(gb_sb, g_sb, b_sb)

        # exp(mask) folded into V and the denominator column:
        # exp(s/8 + m[k]) = exp(s/8) * em[k], and em scales per KEY = per
        # partition of Vaug, so the Exp activation needs no bias operand.
        em_sb = smallp.tile([P, SC], FP32, tag="em")
        nc.scalar.activation(em_sb, amask_sb, AF.Exp)
        ones8 = smallp.tile([P, HPC, 1], BF16, tag="ones8")
        nc.vector.memset(ones8, 1.0)
        ones64 = smallp.tile([P, Dh], BF16, tag="ones64")
        nc.vector.memset(ones64, 1.0)

        # ---- persistent SBUF tensors for projections ----
        Vaug = vpool.tile([P, SC, HPC, Dh + 1], BF16, tag="Vaug")
        QT = qkpool.tile([P, ECH, S], BF16, tag="QT")
        KTp = qkpool.tile([P, HPC, S], BF16, tag="KTp")

        # ---- projection chain emitters (8 accumulation MMs + eviction) ----
        def v_chain(t):
            ps = projp.tile([P, 512], FP32, tag="proj", name=f"psv{t}")
            for c in range(CC):
                nc.tensor.matmul(
                    ps,
                    lhsT=xT_sb[:, c, t * P:(t + 1) * P],
                    rhs=wv_sb[:, c, :],
                    start=(c == 0), stop=(c == CC - 1),
                )
            nc.vector.tensor_scalar(
                Vaug[:, t, :, 0:Dh],
                ps.rearrange("p (h d) -> p h d", h=HPC),
                em_sb[:, t:t + 1], None, OP.mult,
            )
            nc.vector.tensor_scalar(
                Vaug[:, t, :, Dh:Dh + 1], ones8,
                em_sb[:, t:t + 1], None, OP.mult,
            )

        def qt_chain(ec, ss):
            sl = slice(ss * 512, (ss + 1) * 512)
            ps = projp.tile([P, 512], FP32, tag="proj", name=f"psq{ec}_{ss}")
            for c in range(CC):
                nc.tensor.matmul(
                    ps,
                    lhsT=wq_sb[:, c, ec * P:(ec + 1) * P],
                    rhs=xT_sb[:, c, sl],
                    start=(c == 0), stop=(c == CC - 1),
                )
            # (x@W)*g + g*b fused into eviction, cast bf16
            nc.vector.tensor_scalar(
                QT[:, ec, sl], ps,
                gq_sb[:, ec:ec + 1], gbq_sb[:, ec:ec + 1],
                OP.mult, OP.add,
            )

        def kt_chain(ec, ss):
            sl = slice(ss * 512, (ss + 1) * 512)
            ps = projp.tile([P, 512], FP32, tag="proj", name=f"psk{ec}_{ss}")
            for c in range(CC):
                nc.tensor.matmul(
                    ps,
                    lhsT=wk_sb[:, c, ec * P:(ec + 1) * P],
                    rhs=xT_sb[:, c, sl],
                    start=(c == 0), stop=(c == CC - 1),
                )
            # per-head zero-padded to full 128 partitions so score matmuls
            # contract K=128 in the same (128,128) mode as the rest; the pad
            # halves are zeroed here per-chain (cheap bf16 DVE memsets that
            # overlap compute) instead of one giant up-front memset that
            # stalled the first evictions ~16us
            for hi in range(2):
                pp = slice(hi * Dh, (hi + 1) * Dh)
                po = slice((1 - hi) * Dh, (2 - hi) * Dh)
                nc.vector.tensor_scalar(
                    KTp[pp, 2 * ec + hi, sl], ps[pp, :],
                    gk_sb[pp, ec:ec + 1], gbk_sb[pp, ec:ec + 1],
                    OP.mult, OP.add,
                )
                nc.vector.memset(KTp[po, 2 * ec + hi, sl], 0.0)

        # ---- prefix: the minimum projection for the first exps ----
        kt_chain(0, 0)
        qt_chain(0, 0)
        qt_chain(0, 1)

        # remaining chains, interleaved into attention groups' PE slack.
        # group order: heads of one e-chunk first at sp0, then their sp1, so
        # each (KT ec, QT ec/sp) chain is needed as late as possible.
        # group 0 carries the V chains (V t-chunk t is only needed by its
        # lagged ctx matmul) plus KT ec0's remaining s-blocks.
        groups = [(0, 0), (1, 0), (0, 1), (1, 1),
                  (2, 0), (3, 0), (2, 1), (3, 1),
                  (4, 0), (5, 0), (4, 1), (5, 1),
                  (6, 0), (7, 0), (6, 1), (7, 1)]
        # per-group: {iter: [chains to emit after that iter's scores]}
        g0 = {}
        for t in range(8):
            g0[t] = [(lambda tt=2 * t: v_chain(tt)),
                     (lambda tt=2 * t + 1: v_chain(tt))]
        g0[1].append(lambda: kt_chain(0, 1))
        g0[3].append(lambda: kt_chain(0, 2))
        g0[5].append(lambda: kt_chain(0, 3))
        fillers = {
            0: g0,
            1: {2: [lambda: qt_chain(0, 2)], 8: [lambda: qt_chain(0, 3)]},
            2: {2: [lambda: kt_chain(1, 0)], 7: [lambda: kt_chain(1, 1)],
                12: [lambda: kt_chain(1, 2)]},
            3: {2: [lambda: kt_chain(1, 3)], 7: [lambda: qt_chain(1, 0)],
                12: [lambda: qt_chain(1, 1)]},
            4: {4: [lambda: qt_chain(1, 2)]},
            5: {4: [lambda: qt_chain(1, 3)], 10: [lambda: kt_chain(2, 0)]},
            6: {4: [lambda: kt_chain(2, 1)], 10: [lambda: kt_chain(2, 2)]},
            7: {2: [lambda: kt_chain(2, 3)], 7: [lambda: qt_chain(2, 0)],
                12: [lambda: qt_chain(2, 1)]},
            8: {4: [lambda: qt_chain(2, 2)]},
            9: {4: [lambda: qt_chain(2, 3)], 10: [lambda: kt_chain(3, 0)]},
            10: {4: [lambda: kt_chain(3, 1)], 10: [lambda: kt_chain(3, 2)]},
            11: {2: [lambda: kt_chain(3, 3)], 7: [lambda: qt_chain(3, 0)],
                 12: [lambda: qt_chain(3, 1)]},
            12: {4: [lambda: qt_chain(3, 2)]},
            13: {4: [lambda: qt_chain(3, 3)]},
        }

        # ---- attention groups ----
        for g, (h, sp) in enumerate(groups):
            hp, hi = h // 2, h % 2
            fill = fillers.get(g, {})
            ctx_ps = [ctxp.tile([P, 512], FP32, tag="ctx",
                                name=f"ctx{g}_{k}") for k in range(2)]
            ets = {}
            for t in range(SC):
                sps = scp.tile([P, 1024], FP32, tag="sc")
                for j in range(2):
                    q0 = sp * 1024 + j * 512
                    nc.tensor.matmul(
                        sps[:, j * 512:(j + 1) * 512],
                        lhsT=KTp[:, h, t * P:(t + 1) * P],
                        rhs=QT[:, hp, q0:q0 + 512],
                        start=True, stop=True,
                    )
                et = etp.tile([P, 1024], BF16, tag="et")
                nc.scalar.activation(et, sps, AF.Exp)
                ets[t] = et
                # ctx lags scores by one t so PE always has queued work while
                # ACT exponentiates, and exp(t) is never gated on ctx
                if t > 0:
                    for j in range(2):
                        nc.tensor.matmul(
                            ctx_ps[j][0:Dh + 1, :],
                            lhsT=Vaug[:, t - 1, h, :],
                            rhs=ets[t - 1][:, j * 512:(j + 1) * 512],
                            start=(t - 1 == 0), stop=False,
                        )
                    del ets[t - 1]
                for chain in fill.get(t, ()):
                    chain()
            for j in range(2):
                nc.tensor.matmul(
                    ctx_ps[j][0:Dh + 1, :],
                    lhsT=Vaug[:, SC - 1, h, :],
                    rhs=ets[SC - 1][:, j * 512:(j + 1) * 512],
                    start=False, stop=True,
                )

            # ---- normalize + output ----
            # evict PSUM immediately; denominators (row 64 of each j block)
            # are packed onto 32-aligned partitions so (a) the reciprocal
            # runs multi-lane and (b) tiny K=1 ones-stationary matmuls
            # replicate 1/denom across 64 partitions straight into PSUM —
            # no high-latency broadcast DMAs on the tail's critical path.
            cs = []
            for j in range(2):
                c_t = rbp.tile([Dh + 1, 512], FP32, tag="cs")
                nc.vector.tensor_copy(c_t, ctx_ps[j][0:Dh + 1, :])
                cs.append(c_t)
            dpk = dkp.tile([8, P], FP32, tag="dpk")
            for j in range(2):
                nc.sync.dma_start(out=dpk[4 * j:4 * j + 4, :],
                                  in_=cs[j][Dh:Dh + 1, :])
            rct = dkp.tile([8, P], FP32, tag="rct")
            nc.vector.reciprocal(rct, dpk)
            for j in range(2):
                sb = sp * 2 + j
                rcb = rbp.tile([Dh, 512], FP32, tag="rcb")
                for q in range(4):
                    rc = rct[4 * j + q:4 * j + q + 1, :]
                    rc_bcast = bass.AP(
                        tensor=rc.tensor, offset=rc.offset,
                        ap=[list(rc.ap[0]), [0, Dh]] + [list(d) for d in rc.ap[1:]],
                    )
                    nc.sync.dma_start(out=rcb[:, q * P:(q + 1) * P], in_=rc_bcast)
                ot = outp.tile([Dh, 512], FP32, tag="outsb")
                nc.vector.tensor_mul(ot, cs[j][0:Dh, :], rcb)
                nc.vector.tensor_scalar(
                    ot, ot,
                    bvT_sb[hi * Dh:(hi + 1) * Dh, hp:hp + 1], None,
                    OP.add,
                )
                nc.sync.dma_start(
                    out=out[h, :, sb * 512:(sb + 1) * 512], in_=ot)


def _build():
    key = "nc"
    if key in _CACHE:
        return _CACHE[key]
    nc = bacc.Bacc("TRN2", target_bir_lowering=False, debug=False,
                   enable_asserts=False)
    aps = {}

    def din(name, shape, dt):
        aps[name] = nc.dram_tensor(name, shape, dt, kind="ExternalInput").ap()

    din("xT", [P, CC, S], BF16)
    din("wq", [P, CC, E], BF16)
    din("wk", [P, CC, E], BF16)
    din("wv", [P, CC, E], BF16)
    din("wdq", [P, DC, E], BF16)
    din("wdk", [P, DC, E], BF16)
    din("txt", [T, DT], BF16)
    din("tmask", [T, 1], BF16)
    din("amask", [P, SC], FP32)
    din("bq", [P, ECH], FP32)
    din("bk", [P, ECH], FP32)
    din("bv", [P, ECH], FP32)
    din("bdq", [P, ECH], FP32)
    din("bdk", [P, ECH], FP32)
    aps["out"] = nc.dram_tensor("out", [HPC, Dh, S], FP32,
                                kind="ExternalOutput").ap()

    with tile.TileContext(nc) as tc:
        _emit(tc, aps)
    nc.compile()
    _CACHE[key] = nc
    return nc


def kernel(**inputs):
    global last_results
    hs = np.asarray(inputs["hidden_states"], dtype=np.float32)
    amask = np.asarray(inputs["attention_mask"], dtype=np.float32)
    txt = np.asarray(inputs["txt_embedding"], dtype=np.float32)
    tmask = np.asarray(inputs["txt_attention_mask"], dtype=np.float32)
    Wq = np.asarray(inputs["Wq"], dtype=np.float32)
    Wk = np.asarray(inputs["Wk"], dtype=np.float32)
    Wv = np.asarray(inputs["Wv"], dtype=np.float32)
    Wdq = np.asarray(inputs["Wdq"], dtype=np.float32)
    Wdk = np.asarray(inputs["Wdk"], dtype=np.float32)
    bq = np.asarray(inputs["bq"], dtype=np.float32)
    bk = np.asarray(inputs["bk"], dtype=np.float32)
    bv = np.asarray(inputs["bv"], dtype=np.float32)
    bdq = np.asarray(inputs["bdq"], dtype=np.float32)
    bdk = np.asarray(inputs["bdk"], dtype=np.float32)

    nc = _build()

    def pcs(w):
        # [D, e] -> [128, D//128, e]: partition-major so each partition's
        # DMA line is one contiguous block
        d = w.shape[0]
        return np.ascontiguousarray(
            w.reshape(d // P, P, w.shape[1]).transpose(1, 0, 2))

    def pvec(v):
        # [k*128] -> [128, k]
        return np.ascontiguousarray(v.reshape(-1, P).T)

    in_maps = []
    for c in range(NCORES):
        b, g = c // 2, c % 2
        cols = slice(g * E, (g + 1) * E)
        in_maps.append({
            "xT": pcs(np.ascontiguousarray(hs[b].T)).astype(BF16_NP),
            "wq": pcs(Wq[:, cols]).astype(BF16_NP),
            "wk": pcs(Wk[:, cols]).astype(BF16_NP),
            "wv": pcs(Wv[:, cols]).astype(BF16_NP),
            "wdq": pcs(Wdq[:, cols]).astype(BF16_NP),
            "wdk": pcs(Wdk[:, cols]).astype(BF16_NP),
            "txt": txt[b].astype(BF16_NP),
            "tmask": tmask[b].astype(BF16_NP),
            "amask": pvec(amask[b, 0, 0]),
            "bq": pvec(bq[cols]),
            "bk": pvec(bk[cols]),
            "bv": pvec(bv[cols]),
            "bdq": pvec(bdq[cols]),
            "bdk": pvec(bdk[cols]),
        })

    tr = int(os.environ.get("BASS_KERNEL_TRACE", "0"))
    if tr == 2:
        # warm the NEFF (compile+load+run untraced), then trace a second run
        run_bass_kernel_spmd(nc, in_maps, list(range(NCORES)), trace=False)
    res = run_bass_kernel_spmd(nc, in_maps, list(range(NCORES)), trace=bool(tr))
    last_results = res

    outp = np.empty((B, S, DV), dtype=np.float32)
    for c in range(NCORES):
        b, g = c // 2, c % 2
        # device output is ctx^T [head, d, s] -> [s, head*64+d]
        co = res.results[c]["out"].transpose(2, 0, 1).reshape(S, E)
        outp[b, :, g * E:(g + 1) * E] = co
    return outp
